# revision 48
# baseline (speedup 1.0000x reference)
"""Trainium2 Bass kernel for nn_EquivariantBlock (gnn_message_passing).

Single fused kernel, nodes partitioned across 8 cores (2500 each).
Host does sharding/gather/layout only; all FLOPs run on device.

Per core: edges grouped by destination window (128-node windows, node->window
assignment load-balanced via LPT so every window needs ~the same tile count).
Edge phase per 128-edge tile: bf16 edge MLP on PE -> per-edge TP weights in
PSUM (never touch HBM; W3 columns pre-permuted + CG/alpha scales folded on
host) -> ACT evacuates weights to SBUF bf16 -> DVE forms per-edge products
(plain tensor_tensor, 2x bf16 mode) -> Pool engine reduces -> one-hot scatter
matmul accumulates per-window sums in PSUM.  Node phase interleaved per
window: degree-mean, self-interaction + gate matmuls (bf16), batch-stat slab.
End phase: stats matmul -> 640B AllReduce across the 8 cores -> BN scale/shift
vectors on-device -> broadcast via matmul -> gated residual update -> one DMA.
"""

import numpy as np

MUL0, MUL1 = 32, 16
EDGE_DIM, HID = 32, 64
WNUM = 2304
N_NODES, N_EDGES = 20000, 100000
EPS = 1e-5
ALPHA = 1.0 / np.sqrt(48.0)
INV_SQRT3 = 1.0 / np.sqrt(3.0)

N_CORES = 8
NPC = N_NODES // N_CORES          # nodes per core = 2500
N_WIN = 20                        # windows per core
NPW = NPC // N_WIN                # nodes per window = 125 (< 128)
WIN = 128                         # window slot size (partition dim)
NPAD = N_WIN * WIN                # 2560
ET = 128                          # edges per tile

_last_exec_ns = None
_last_rr = []

# column permutation: reference h layout [s(32), v (i-major: i*3+c)]
# internal layout  [s(32), v (c-major: c*16+i)]
_perm_ref2int = np.concatenate(
    [np.arange(32)] + [32 + np.arange(16) * 3 + c for c in range(3)]
).astype(np.int64)
_perm_int2ref = np.argsort(_perm_ref2int)


def _w3_permute(W3b):
    """Permute + scale the (W3;b3) columns for the on-device TP layout.

    Input rows [65, 2304] in reference order:
      w00 col i*32+o, w10 1024+i*32+o, w01 1536+i*16+o, w11 2048+i*16+o.

    Output layout, scales folded.  Blocks 00/10/11 use a k-interleaved
    order (i = k*8 + r) so the i-contraction runs as flat contiguous
    half-adds on DVE followed by an 8-wide window-level reduce:
      00 region       k*256 + o*8 + r (k<4,o<32,r<8): alpha*w00[k*8+r, o]
      10 region 1024+ k*256 + o*8 + r (k<2,o<32,r<8): a/sqrt3*w10[k*8+r,o]
      01 region 1536+ o*32 + i       (o<16,i<32):     alpha*w01[i,o]
      11 region 2048+ k*128 + o*8 + r (k<2,o<16,r<8): alpha*w11[k*8+r,o]
    """
    idx = np.empty(WNUM, np.int64)
    scl = np.empty(WNUM, np.float32)
    for k in range(4):
        for o in range(32):
            for r in range(8):
                idx[k * 256 + o * 8 + r] = (k * 8 + r) * 32 + o
                scl[k * 256 + o * 8 + r] = ALPHA
    for k in range(2):
        for o in range(32):
            for r in range(8):
                idx[1024 + k * 256 + o * 8 + r] = 1024 + (k * 8 + r) * 32 + o
                scl[1024 + k * 256 + o * 8 + r] = ALPHA * INV_SQRT3
    for o in range(16):
        for i in range(32):
            idx[1536 + o * 32 + i] = 1536 + i * 16 + o
            scl[1536 + o * 32 + i] = ALPHA
    for k in range(2):
        for o in range(16):
            for r in range(8):
                idx[2048 + k * 128 + o * 8 + r] = 2048 + (k * 8 + r) * 16 + o
                scl[2048 + k * 128 + o * 8 + r] = ALPHA
    return (W3b[:, idx] * scl[None, :]).astype(np.float32)


def _balance_nodes(dst):
    """Assign each core's local nodes to N_WIN windows (<=NPW nodes each),
    balancing per-window edge counts (greedy LPT).  Returns win_of, pos_of
    [N_CORES, NPC] and per-(core,window) edge counts."""
    core = dst // NPC
    dloc = dst - core * NPC
    win_of = np.zeros((N_CORES, NPC), np.int64)
    pos_of = np.zeros((N_CORES, NPC), np.int64)
    ecnt = np.zeros((N_CORES, N_WIN), np.int64)
    for c in range(N_CORES):
        deg = np.bincount(dloc[core == c], minlength=NPC)
        order = np.argsort(-deg, kind="stable")
        loads = np.zeros(N_WIN, np.int64)
        counts = np.zeros(N_WIN, np.int64)
        for n in order:
            open_w = np.nonzero(counts < NPW)[0]
            w = open_w[np.argmin(loads[open_w])]
            win_of[c, n] = w
            pos_of[c, n] = counts[w]
            counts[w] += 1
            loads[w] += deg[n]
        ecnt[c] = loads
    return win_of, pos_of, ecnt


def _shard(edge_index):
    src, dst = edge_index[0], edge_index[1]
    win_of, pos_of, ecnt = _balance_nodes(dst)
    tiles_per_win = [
        max(1, int(max((ecnt[c, w] + ET - 1) // ET for c in range(N_CORES))))
        for w in range(N_WIN)
    ]
    core = dst // NPC
    dloc = dst - core * NPC
    order = [[None] * N_WIN for _ in range(N_CORES)]
    for c in range(N_CORES):
        idx = np.nonzero(core == c)[0]
        w_of = win_of[c][dloc[idx]]
        s = np.argsort(w_of, kind="stable")
        idx = idx[s]
        w_of = w_of[s]
        bounds = np.searchsorted(w_of, np.arange(N_WIN + 1))
        for w in range(N_WIN):
            order[c][w] = idx[bounds[w]:bounds[w + 1]]
    return order, tiles_per_win, win_of, pos_of


def _build_core_inputs(inputs, order, tiles_per_win, win_of, pos_of, c, bf16):
    h = inputs["h"]
    edge_sh = inputs["edge_sh"]
    ef = inputs["edge_features"]
    src = inputs["edge_index"][0]
    dst = inputs["edge_index"][1]

    T = int(sum(tiles_per_win))
    E_pad = T * ET
    svx = np.zeros((E_pad, 96), np.float32)
    efp = np.zeros((E_pad, EDGE_DIM), np.float32)

    pos = 0
    for w in range(N_WIN):
        ids = order[c][w]
        n = len(ids)
        sl = slice(pos, pos + n)
        hs = h[src[ids]][:, _perm_ref2int]     # [n, 80] internal layout
        svx[sl, 0:80] = hs
        svx[sl, 80] = edge_sh[ids, 0]
        svx[sl, 81:84] = edge_sh[ids, 1:4]
        svx[sl, 84] = 1.0
        svx[sl, 85] = -pos_of[c][dst[ids] - c * NPC].astype(np.float32)
        efp[sl] = ef[ids]
        pos += tiles_per_win[w] * ET
    efT = np.ascontiguousarray(
        efp.reshape(T, ET, EDGE_DIM).transpose(0, 2, 1)
    )

    # node-side layouts follow the per-core window permutation
    hsl = h[c * NPC:(c + 1) * NPC][:, _perm_ref2int]   # [2500, 80] internal
    col = win_of[c] * WIN + pos_of[c]                  # node -> slab column
    hT_s = np.zeros((33, NPAD), np.float32)
    hT_s[0, col] = 1.0
    hT_s[1:, col] = hsl[:, :32].T
    hT_v = [np.zeros((16, NPAD), np.float32) for _ in range(3)]
    for cc in range(3):
        hT_v[cc][:, col] = hsl[:, 32 + cc * 16:32 + (cc + 1) * 16].T
    h_nm = np.zeros((WIN, N_WIN, 80), np.float32)      # node-major slab
    h_nm[pos_of[c], win_of[c]] = hsl

    return dict(
        svx=svx.reshape(T, ET, 96).astype(bf16),
        efT=efT.astype(bf16),
        hT_s=hT_s.astype(bf16),
        hT_v0=hT_v[0].astype(bf16),
        hT_v1=hT_v[1].astype(bf16),
        hT_v2=hT_v[2].astype(bf16),
        h_nm=h_nm,
    ), T


def _fused_bass(T, tiles_per_win):
    import concourse.bacc as bacc
    import concourse.mybir as mybir
    import concourse.tile as tile

    fp32 = mybir.dt.float32
    bf16 = mybir.dt.bfloat16
    Alu = mybir.AluOpType
    Act = mybir.ActivationFunctionType
    X = mybir.AxisListType.X

    nc = bacc.Bacc("TRN2", target_bir_lowering=False, debug=False,
                   num_devices=N_CORES)
    d_svx = nc.dram_tensor("svx", [T, ET, 96], bf16, kind="ExternalInput")
    d_efT = nc.dram_tensor("efT", [T, 32, ET], bf16, kind="ExternalInput")
    d_hTs = nc.dram_tensor("hT_s", [33, NPAD], bf16, kind="ExternalInput")
    d_hTv = [
        nc.dram_tensor(f"hT_v{i}", [16, NPAD], bf16, kind="ExternalInput")
        for i in range(3)
    ]
    d_hnm = nc.dram_tensor("h_nm", [WIN, N_WIN, 80], fp32,
                           kind="ExternalInput")
    d_W1 = nc.dram_tensor("W1", [32, 64], bf16, kind="ExternalInput")
    d_b1 = nc.dram_tensor("b1", [64, 1], fp32, kind="ExternalInput")
    d_W2 = nc.dram_tensor("W2", [64, 64], bf16, kind="ExternalInput")
    d_b2 = nc.dram_tensor("b2", [64, 1], fp32, kind="ExternalInput")
    d_W3 = nc.dram_tensor("W3", [65, WNUM], bf16, kind="ExternalInput")
    d_Wn = nc.dram_tensor("Wn", [33, 64], bf16, kind="ExternalInput")
    d_Wv = nc.dram_tensor("Wv", [16, 32], bf16, kind="ExternalInput")
    d_bnw = nc.dram_tensor("bnw", [1, 80], fp32, kind="ExternalInput")
    d_bnb = nc.dram_tensor("bnb", [1, 80], fp32, kind="ExternalInput")
    d_out = nc.dram_tensor("out", [WIN, N_WIN, 80], fp32,
                           kind="ExternalOutput")

    with tile.TileContext(nc) as tc, \
            nc.allow_low_precision(reason="bf16 TP well within 2e-2 tol"):
        with tc.tile_pool(name="singles", bufs=1) as singles:
            sW1 = singles.tile([32, 64], bf16)
            nc.sync.dma_start(out=sW1, in_=d_W1[:, :])
            sb1 = singles.tile([64, 1], fp32)
            nc.sync.dma_start(out=sb1, in_=d_b1[:, :])
            sW2 = singles.tile([64, 64], bf16)
            nc.sync.dma_start(out=sW2, in_=d_W2[:, :])
            sb2 = singles.tile([64, 1], fp32)
            nc.sync.dma_start(out=sb2, in_=d_b2[:, :])
            sW3 = singles.tile([65, WNUM], bf16)
            nc.sync.dma_start(out=sW3, in_=d_W3[:, :])
            sWn = singles.tile([33, 64], bf16)
            nc.sync.dma_start(out=sWn, in_=d_Wn[:, :])
            sWv = singles.tile([16, 32], bf16)
            nc.sync.dma_start(out=sWv, in_=d_Wv[:, :])
            sHs = singles.tile([33, NPAD], bf16)
            nc.sync.dma_start(out=sHs, in_=d_hTs[:, :])
            sHv = []
            for i in range(3):
                t = singles.tile([16, NPAD], bf16, tag=f"hv{i}")
                nc.sync.dma_start(out=t, in_=d_hTv[i][:, :])
                sHv.append(t)
            # h_nm is only read in the end phase; DMA issued there so the
            # 820KB transfer doesn't stall the first edge tiles' loads
            h_nm = singles.tile([WIN, N_WIN, 80], fp32)
            bnw = singles.tile([1, 80], fp32)
            nc.sync.dma_start(out=bnw, in_=d_bnw[:, :])
            bnb = singles.tile([1, 80], fp32)
            nc.sync.dma_start(out=bnb, in_=d_bnb[:, :])

            iotaI = singles.tile([ET, WIN], mybir.dt.int32)
            nc.gpsimd.iota(iotaI, [[1, WIN]], channel_multiplier=0)
            iotaB = singles.tile([ET, WIN], bf16)
            nc.vector.tensor_copy(iotaB, iotaI)
            ones_col = singles.tile([WIN, 1], bf16)
            nc.vector.memset(ones_col, 1.0)
            ones_row = singles.tile([1, WIN], fp32)
            nc.vector.memset(ones_row, 1.0)

            slab_st = singles.tile([WIN, N_WIN, 160], bf16)
            slab_g = singles.tile([WIN, N_WIN, 80], bf16)
            slab_out = singles.tile([WIN, N_WIN, 80], fp32)

            # manual double-buffer for x2a: row 64 is a constant ones row
            # (the b3 contraction row), written once — pools would force a
            # per-tile rewrite.
            x2a_bufs = []
            for i in range(2):
                xt = singles.tile([65, ET], bf16, tag=f"x2a{i}")
                nc.vector.memset(xt[64:65, :], 1.0)
                x2a_bufs.append(xt)

            # ---------------- edge + node phase (per window) ----------------
            # scatter payload layout (945 bf16 cols per edge):
            #   0:256    00-block partial sums, (o=32, r=8)
            #   256:512  10-block partial sums, (o=32, r=8)
            #   512:896  11-block partial sums, (c=3, o=16, r=8)
            #   896:944  p01*sh1, (c=3, o=16)
            #   944      valid (degree)
            with (
                tc.tile_pool(name="edma", bufs=6) as edma,
                tc.tile_pool(name="esb", bufs=3) as esb,
                tc.tile_pool(name="nsb", bufs=2) as nsb,
                tc.tile_pool(name="wpsA", bufs=1, space="PSUM") as wpsA,
                tc.tile_pool(name="wps01", bufs=1, space="PSUM") as wps01,
                tc.tile_pool(name="wpsB", bufs=1, space="PSUM") as wpsB,
                tc.tile_pool(name="mmout", bufs=1, space="PSUM") as mmout,
                tc.tile_pool(name="sps", bufs=1, space="PSUM") as sps,
            ):
                # self-interaction + gate pre-acts are hoisted off the
                # per-window critical path (they depend only on constant
                # slabs), but staggered one window per tile-group so the
                # 20-window block doesn't serialize ahead of the first edge
                # tile through the shared mmout bank.  Layout per window:
                # [si_s(32) | gate_s(32) | 3 x (si_v(16) | gate_v(16))]
                slab_nv = singles.tile([WIN, N_WIN, 160], fp32)

                def emit_node_mm(w):
                    nmo = mmout.tile([128, 160], fp32, tag="mm")
                    nc.tensor.matmul(nmo[:, 0:64],
                                     sHs[:, w * WIN:(w + 1) * WIN], sWn,
                                     start=True, stop=True)
                    for cc in range(3):
                        nc.tensor.matmul(
                            nmo[:, 64 + cc * 32:96 + cc * 32],
                            sHv[cc][:, w * WIN:(w + 1) * WIN], sWv,
                            start=True, stop=True)
                    nc.scalar.activation(slab_nv[:, w, :], nmo, Act.Copy)

                t_idx = 0
                for w in range(N_WIN):
                    emit_node_mm(w)
                    ps_sum = sps.tile([WIN, 945], fp32, tag="scat")
                    jlast = tiles_per_win[w] - 1
                    for j in range(tiles_per_win[w]):
                        t = t_idx
                        t_idx += 1
                        sv = edma.tile([ET, 96], bf16, tag="svx")
                        nc.sync.dma_start(out=sv, in_=d_svx[t, :, :])
                        ef_t = edma.tile([32, ET], bf16, tag="ef")
                        nc.sync.dma_start(out=ef_t, in_=d_efT[t, :, :])

                        # --- edge MLP (feature-major, bf16) ---
                        mo1 = mmout.tile([128, 160], fp32, tag="mm")
                        nc.tensor.matmul(mo1[0:64, 0:128], sW1, ef_t,
                                         start=True, stop=True)
                        x1 = esb.tile([64, ET], bf16, tag="x1")
                        nc.scalar.activation(x1, mo1[0:64, 0:128], Act.Silu,
                                             bias=sb1)
                        mo2 = mmout.tile([128, 160], fp32, tag="mm")
                        nc.tensor.matmul(mo2[0:64, 0:128], sW2, x1,
                                         start=True, stop=True)
                        x2a = x2a_bufs[t % 2]
                        nc.scalar.activation(x2a[0:64, :], mo2[0:64, 0:128],
                                             Act.Silu, bias=sb2)

                        # --- mm3: per-edge TP weights, 3 PSUM regions ---
                        psA = wpsA.tile([ET, 1536], fp32, tag="A")
                        for c0 in (0, 512, 1024):
                            nc.tensor.matmul(psA[:, c0:c0 + 512], x2a,
                                             sW3[:, c0:c0 + 512],
                                             start=True, stop=True)
                        ps01 = wps01.tile([ET, 512], fp32, tag="o1")
                        nc.tensor.matmul(ps01, x2a, sW3[:, 1536:2048],
                                         start=True, stop=True)
                        psB = wpsB.tile([ET, 256], fp32, tag="B")
                        nc.tensor.matmul(psB, x2a, sW3[:, 2048:2304],
                                         start=True, stop=True)

                        # --- per-edge features ---
                        # fp32 copies of the per-edge scalars (ts needs f32)
                        aux32 = esb.tile([ET, 6], fp32, tag="aux32")
                        nc.gpsimd.tensor_copy(aux32, sv[:, 80:86])
                        # fAV = [se*sh0 (32) | dv (16) | vec*sh0 (48)]
                        # scale-by-partition-scalar runs on ACT (idle)
                        fAV = esb.tile([ET, 96], bf16, tag="fAV")
                        nc.scalar.activation(
                            fAV[:, 0:32], sv[:, 0:32], Act.Copy,
                            scale=aux32[:, 0:1])
                        nc.scalar.activation(
                            fAV[:, 48:96], sv[:, 32:80], Act.Copy,
                            scale=aux32[:, 0:1])
                        t3 = esb.tile([ET, 48], bf16, tag="t3")
                        nc.gpsimd.tensor_tensor(
                            out=t3, in0=sv[:, 32:80],
                            in1=sv[:, 81:84].unsqueeze(2).broadcast_to(
                                (ET, 3, 16)),
                            op=Alu.mult)
                        nc.vector.tensor_reduce(
                            out=fAV[:, 32:48],
                            in_=t3.rearrange("p (c i) -> p i c", c=3),
                            axis=X, op=Alu.add)

                        # --- TP products: DVE straight from PSUM ---
                        prod00 = esb.tile([ET, 1024], bf16, tag="prod00")
                        nc.vector.tensor_tensor(
                            out=prod00.rearrange("p (k o r) -> p k o r",
                                                 k=4, o=32),
                            in0=psA[:, 0:1024].rearrange(
                                "p (k o r) -> p k o r", k=4, o=32),
                            in1=fAV[:, 0:32]
                                .rearrange("p (k r) -> p k r", k=4)
                                .unsqueeze(2).broadcast_to((ET, 4, 32, 8)),
                            op=Alu.mult)
                        prod10 = esb.tile([ET, 512], bf16, tag="prod10")
                        nc.vector.tensor_tensor(
                            out=prod10.rearrange("p (k o r) -> p k o r",
                                                 k=2, o=32),
                            in0=psA[:, 1024:1536].rearrange(
                                "p (k o r) -> p k o r", k=2, o=32),
                            in1=fAV[:, 32:48]
                                .rearrange("p (k r) -> p k r", k=2)
                                .unsqueeze(2).broadcast_to((ET, 2, 32, 8)),
                            op=Alu.mult)
                        fVv = fAV[:, 48:96].rearrange("p (c i) -> p c i",
                                                      c=3)
                        prodBk = []
                        for k in range(2):
                            pk = esb.tile([ET, 384], bf16, tag=f"prodB{k}")
                            nc.vector.tensor_tensor(
                                out=pk.rearrange("p (c o r) -> p c o r",
                                                 c=3, o=16),
                                in0=psB[:, k * 128:(k + 1) * 128]
                                    .rearrange("p (o r) -> p o r", o=16)
                                    .unsqueeze(1)
                                    .broadcast_to((ET, 3, 16, 8)),
                                in1=fVv[:, :, k * 8:(k + 1) * 8]
                                    .unsqueeze(2)
                                    .broadcast_to((ET, 3, 16, 8)),
                                op=Alu.mult)
                            prodBk.append(pk)
                        w01 = esb.tile([ET, 512], bf16, tag="w01")
                        nc.scalar.activation(w01, ps01, Act.Copy)
                        prod01 = esb.tile([ET, 512], bf16, tag="prod01")
                        nc.gpsimd.tensor_tensor(
                            out=prod01.rearrange("p (o i) -> p o i", o=16),
                            in0=w01.rearrange("p (o i) -> p o i", o=16),
                            in1=sv[:, 0:32].unsqueeze(1).broadcast_to(
                                (ET, 16, 32)),
                            op=Alu.mult)

                        # --- fold k-halves: flat contiguous adds ---
                        msgw = esb.tile([ET, 945], bf16, tag="msgw")
                        t00 = esb.tile([ET, 512], bf16, tag="t00")
                        nc.vector.tensor_tensor(
                            out=t00, in0=prod00[:, 0:512],
                            in1=prod00[:, 512:1024], op=Alu.add)
                        nc.vector.tensor_tensor(
                            out=msgw[:, 0:256], in0=t00[:, 0:256],
                            in1=t00[:, 256:512], op=Alu.add)
                        nc.gpsimd.tensor_tensor(
                            out=msgw[:, 256:512], in0=prod10[:, 0:256],
                            in1=prod10[:, 256:512], op=Alu.add)
                        nc.gpsimd.tensor_tensor(
                            out=msgw[:, 512:896], in0=prodBk[0],
                            in1=prodBk[1], op=Alu.add)

                        # --- 01-block: full reduce + sh1 outer product ---
                        p01 = esb.tile([ET, 16], bf16, tag="p01")
                        nc.vector.tensor_reduce(
                            out=p01,
                            in_=prod01.rearrange("p (o i) -> p o i", o=16),
                            axis=X, op=Alu.add)
                        nc.gpsimd.tensor_tensor(
                            out=msgw[:, 896:944].rearrange(
                                "p (c o) -> p c o", c=3),
                            in0=p01.unsqueeze(1).broadcast_to((ET, 3, 16)),
                            in1=sv[:, 81:84].unsqueeze(2).broadcast_to(
                                (ET, 3, 16)),
                            op=Alu.mult)
                        nc.gpsimd.tensor_copy(msgw[:, 944:945], sv[:, 84:85])

                        # --- one-hot scatter matmul (moving dim <= 512) ---
                        # one-hot on ACT: relu(1 - |iota - dstw|); svx col 85
                        # holds -dstw so it can ride the activation bias
                        absd = esb.tile([ET, WIN], bf16, tag="absd")
                        nc.scalar.activation(absd, iotaB, Act.Abs,
                                             bias=aux32[:, 5:6])
                        S = esb.tile([ET, WIN], bf16, tag="S")
                        nc.scalar.activation(S, absd, Act.Relu,
                                             scale=-1.0, bias=1.0)
                        for c0, c1 in ((0, 512), (512, 945)):
                            nc.tensor.matmul(
                                ps_sum[:, c0:c1], S, msgw[:, c0:c1],
                                start=(j == 0), stop=(j == jlast),
                                skip_group_check=True)

                    # ---------------- node phase for window w ----------------
                    # window-level reduce of the scattered 8-wide partials
                    tw = nsb.tile([WIN, 112], fp32, tag="tw")
                    nc.vector.tensor_reduce(
                        out=tw,
                        in_=ps_sum[:, 0:896].rearrange(
                            "p (g r) -> p g r", g=112),
                        axis=X, op=Alu.add)
                    summed = nsb.tile([WIN, 80], fp32, tag="summed")
                    nc.vector.tensor_tensor(
                        out=summed[:, 0:32], in0=tw[:, 0:32],
                        in1=tw[:, 32:64], op=Alu.add)
                    nc.vector.tensor_tensor(
                        out=summed[:, 32:80], in0=tw[:, 64:112],
                        in1=ps_sum[:, 896:944], op=Alu.add)
                    degc = nsb.tile([WIN, 1], fp32, tag="degc")
                    nc.vector.tensor_scalar(
                        degc, ps_sum[:, 944:945], 1.0, None, op0=Alu.max)
                    rdeg = nsb.tile([WIN, 1], fp32, tag="rdeg")
                    nc.vector.reciprocal(rdeg, degc)
                    agg = nsb.tile([WIN, 80], fp32, tag="agg")
                    nc.vector.tensor_scalar(
                        agg, summed, rdeg, None, op0=Alu.mult)

                    # upd -> slab_st[:, w, 0:80]; sq -> [:, w, 80:160]
                    # (self-interaction comes from the SBUF slab, so these
                    # run on Pool; sigmoids are batched in the end phase)
                    nv_v = slab_nv[:, w, 64:160].rearrange(
                        "p (c k) -> p c k", c=3)
                    nc.gpsimd.tensor_tensor(
                        out=slab_st[:, w, 0:32], in0=agg[:, 0:32],
                        in1=slab_nv[:, w, 0:32], op=Alu.add)
                    nc.gpsimd.tensor_tensor(
                        out=slab_st[:, w, 32:80].rearrange(
                            "p (c i) -> p c i", c=3),
                        in0=agg[:, 32:80].rearrange("p (c i) -> p c i", c=3),
                        in1=nv_v[:, :, 0:16], op=Alu.add)
                    nc.gpsimd.tensor_tensor(
                        out=slab_st[:, w, 80:160],
                        in0=slab_st[:, w, 0:80],
                        in1=slab_st[:, w, 0:80], op=Alu.mult)

            # ---------------- end phase: stats, allreduce, BN, update -------
            with (
                tc.tile_pool(name="eps", bufs=1, space="PSUM") as eps_p,
                tc.tile_pool(name="fsb", bufs=1) as fsb,
                tc.tile_pool(name="dram", bufs=2, space="DRAM") as dram,
            ):
                nc.sync.dma_start(out=h_nm, in_=d_hnm[:, :, :])
                # all gate sigmoids in two batched instrs (one table load)
                nc.scalar.activation(
                    slab_g[:, :, 0:32], slab_nv[:, :, 32:64], Act.Sigmoid)
                nc.scalar.activation(
                    slab_g[:, :, 32:80].rearrange(
                        "p w (c i) -> p w c i", c=3),
                    slab_nv[:, :, 64:160].rearrange(
                        "p w (c k) -> p w c k", c=3)[:, :, :, 16:32],
                    Act.Sigmoid)
                ps_st = eps_p.tile([1, 160], fp32, tag="st")
                for w in range(N_WIN):
                    nc.tensor.matmul(
                        ps_st, ones_col, slab_st[:, w, :],
                        start=(w == 0), stop=(w == N_WIN - 1),
                        skip_group_check=True)
                st_sb = fsb.tile([1, 160], fp32, tag="stsb")
                nc.scalar.activation(st_sb, ps_st, Act.Copy)

                ib = dram.tile([1, 160], fp32, tag="ib")
                ob = dram.tile([1, 160], fp32, tag="ob")
                nc.gpsimd.dma_start(ib[:], st_sb[:])
                nc.gpsimd.collective_compute(
                    "AllReduce", mybir.AluOpType.add,
                    replica_groups=[list(range(N_CORES))],
                    ins=[ib.opt()], outs=[ob.opt()])
                st_r = fsb.tile([1, 160], fp32, tag="str")
                nc.gpsimd.dma_start(st_r[:], ob[:])

                inv_n = 1.0 / float(N_NODES)
                meanb = fsb.tile([1, 80], fp32, tag="meanb")
                nc.vector.tensor_scalar(
                    meanb, st_r[:, 0:80], inv_n, None, op0=Alu.mult)
                nc.vector.memset(meanb[:, 32:80], 0.0)
                ex2 = fsb.tile([1, 80], fp32, tag="ex2")
                nc.vector.tensor_scalar(
                    ex2, st_r[:, 80:160], inv_n, None, op0=Alu.mult)
                m2 = fsb.tile([1, 80], fp32, tag="m2")
                nc.vector.tensor_tensor(out=m2, in0=meanb, in1=meanb,
                                        op=Alu.mult)
                exm = fsb.tile([1, 80], fp32, tag="exm")
                nc.vector.tensor_tensor(out=exm, in0=ex2, in1=m2,
                                        op=Alu.subtract)
                vn = fsb.tile([1, 16], fp32, tag="vn")
                nc.vector.tensor_reduce(
                    out=vn,
                    in_=exm[:, 32:80].rearrange("p (c i) -> p i c", c=3),
                    axis=X, op=Alu.add)
                varb = fsb.tile([1, 80], fp32, tag="varb")
                nc.vector.tensor_scalar(
                    varb[:, 0:32], exm[:, 0:32], 1.0, float(EPS),
                    op0=Alu.mult, op1=Alu.add)
                nc.vector.tensor_scalar(
                    varb[:, 32:80].rearrange("p (c i) -> p c i", c=3),
                    vn.unsqueeze(1).broadcast_to((1, 3, 16)),
                    1.0 / 3.0, float(EPS), op0=Alu.mult, op1=Alu.add)
                rec = fsb.tile([1, 80], fp32, tag="rec")
                nc.vector.reciprocal(rec, varb)
                rstd = fsb.tile([1, 80], fp32, tag="rstd")
                nc.scalar.activation(rstd, rec, Act.Sqrt)
                scsh = fsb.tile([1, 160], fp32, tag="scsh")
                nc.vector.tensor_tensor(
                    out=scsh[:, 0:80], in0=rstd, in1=bnw, op=Alu.mult)
                msc = fsb.tile([1, 80], fp32, tag="msc")
                nc.vector.tensor_tensor(
                    out=msc, in0=meanb, in1=scsh[:, 0:80], op=Alu.mult)
                nc.vector.tensor_tensor(
                    out=scsh[:, 80:160], in0=bnb, in1=msc, op=Alu.subtract)

                ps_b = eps_p.tile([128, 160], fp32, tag="bc")
                nc.tensor.matmul(ps_b, ones_row, scsh, start=True, stop=True)
                scshB = fsb.tile([128, 160], fp32, tag="scshB")
                nc.scalar.activation(scshB, ps_b, Act.Copy)

                # batched gated residual update over the whole node slab:
                # out = (upd*sc + sh) * g + h, broadcasting sc/sh per window
                scB = scshB[:, 0:80].unsqueeze(1).broadcast_to(
                    (WIN, N_WIN, 80))
                shB = scshB[:, 80:160].unsqueeze(1).broadcast_to(
                    (WIN, N_WIN, 80))
                t1 = fsb.tile([WIN, N_WIN, 80], fp32, tag="t1")
                nc.vector.tensor_tensor(
                    out=t1, in0=slab_st[:, :, 0:80], in1=scB, op=Alu.mult)
                t2 = fsb.tile([WIN, N_WIN, 80], fp32, tag="t2")
                nc.gpsimd.tensor_tensor(
                    out=t2, in0=t1, in1=shB, op=Alu.add)
                nc.vector.tensor_tensor(
                    out=t1, in0=t2, in1=slab_g, op=Alu.mult)
                nc.gpsimd.tensor_tensor(
                    out=slab_out, in0=t1, in1=h_nm, op=Alu.add)
                nc.sync.dma_start(out=d_out[:, :, :], in_=slab_out)
    nc.compile()
    return nc


def kernel(**inputs):
    import os
    from concourse.bass_utils import run_bass_kernel_spmd
    import ml_dtypes

    bf16 = ml_dtypes.bfloat16
    trace = os.environ.get("KERNEL_TRACE", "0") == "1"
    inputs = {k: np.asarray(v) for k, v in inputs.items()}
    edge_index = inputs["edge_index"].astype(np.int64)
    inputs["edge_index"] = edge_index
    for k in list(inputs):
        if inputs[k].dtype == np.float64:
            inputs[k] = inputs[k].astype(np.float32)

    order, tiles_per_win, win_of, pos_of = _shard(edge_index)

    # shared weights (host-side dtype conversion / packing)
    W3b = np.vstack([
        inputs["W3"].astype(np.float32),
        inputs["b3"].astype(np.float32).reshape(1, WNUM),
    ])
    W3p = _w3_permute(W3b).astype(bf16)
    Wn = np.zeros((33, 64), np.float32)
    Wn[0, 0:32] = inputs["bs_s"]
    Wn[1:, 0:32] = inputs["ws_s"]
    Wn[0, 32:64] = inputs["bg_s"]
    Wn[1:, 32:64] = inputs["wg_s"]
    Wv = np.zeros((16, 32), np.float32)
    Wv[:, 0:16] = inputs["ws_v"]
    Wv[:, 16:32] = inputs["wg_v"]
    bnw_row = np.zeros((1, 80), np.float32)
    bnw_row[0, 0:32] = inputs["bn_ws"]
    for cc in range(3):
        bnw_row[0, 32 + cc * 16:48 + cc * 16] = inputs["bn_wv"]
    bnb_row = np.zeros((1, 80), np.float32)
    bnb_row[0, 0:32] = inputs["bn_bs"]

    core_maps = []
    T = None
    for c in range(N_CORES):
        m, T = _build_core_inputs(
            inputs, order, tiles_per_win, win_of, pos_of, c, bf16)
        m["W1"] = inputs["W1"].astype(bf16)
        m["b1"] = inputs["b1"].astype(np.float32).reshape(64, 1)
        m["W2"] = inputs["W2"].astype(bf16)
        m["b2"] = inputs["b2"].astype(np.float32).reshape(64, 1)
        m["W3"] = W3p
        m["Wn"] = Wn.astype(bf16)
        m["Wv"] = Wv.astype(bf16)
        m["bnw"] = bnw_row
        m["bnb"] = bnb_row
        core_maps.append(m)

    nc1 = _fused_bass(T, tiles_per_win)
    r1 = run_bass_kernel_spmd(
        nc1, core_maps, core_ids=list(range(N_CORES)), trace=trace)
    global _last_exec_ns
    _last_exec_ns = r1.exec_time_ns
    _last_rr.clear()
    _last_rr.append(r1)

    out = np.zeros((N_NODES, 80), np.float32)
    for c in range(N_CORES):
        slab = r1.results[c]["out"]          # [WIN, N_WIN, 80]
        blk = slab[pos_of[c], win_of[c]]     # [2500, 80] internal layout
        out[c * NPC:(c + 1) * NPC] = blk[:, _perm_int2ref]
    return out


if __name__ == "__main__":
    import reference

    inp = reference.setup_inputs()
    inp = {k: np.asarray(v) for k, v in inp.items()}
    expected = np.asarray(reference.reference(**inp))
    actual = kernel(**inp)
    err = np.abs(actual - expected)
    rel = np.linalg.norm(actual - expected) / np.linalg.norm(expected)
    print("max abs err:", err.max(), "rel:", rel)


# revision 49
# speedup vs baseline: 8.4725x; 8.4725x over previous
"""Trainium2 Bass kernel for nn_EquivariantBlock (gnn_message_passing).

Single fused kernel, nodes partitioned across 8 cores (2500 each).
Host does sharding/gather/layout only; all FLOPs run on device.

Per core: edges grouped by destination window (128-node windows, node->window
assignment load-balanced via LPT so every window needs ~the same tile count).
Edge phase per 128-edge tile: bf16 edge MLP on PE -> per-edge TP weights in
PSUM (never touch HBM; W3 columns pre-permuted + CG/alpha scales folded on
host) -> ACT evacuates weights to SBUF bf16 -> DVE forms per-edge products
(plain tensor_tensor, 2x bf16 mode) -> Pool engine reduces -> one-hot scatter
matmul accumulates per-window sums in PSUM.  Node phase interleaved per
window: degree-mean, self-interaction + gate matmuls (bf16), batch-stat slab.
End phase: stats matmul -> 640B AllReduce across the 8 cores -> BN scale/shift
vectors on-device -> broadcast via matmul -> gated residual update -> one DMA.
"""

import numpy as np

MUL0, MUL1 = 32, 16
EDGE_DIM, HID = 32, 64
WNUM = 2304
N_NODES, N_EDGES = 20000, 100000
EPS = 1e-5
ALPHA = 1.0 / np.sqrt(48.0)
INV_SQRT3 = 1.0 / np.sqrt(3.0)

N_CORES = 8
NPC = N_NODES // N_CORES          # nodes per core = 2500
N_WIN = 20                        # windows per core
NPW = NPC // N_WIN                # nodes per window = 125 (< 128)
WIN = 128                         # window slot size (partition dim)
NPAD = N_WIN * WIN                # 2560
ET = 128                          # edges per tile

_last_exec_ns = None
_last_rr = []

# column permutation: reference h layout [s(32), v (i-major: i*3+c)]
# internal layout  [s(32), v (c-major: c*16+i)]
_perm_ref2int = np.concatenate(
    [np.arange(32)] + [32 + np.arange(16) * 3 + c for c in range(3)]
).astype(np.int64)
_perm_int2ref = np.argsort(_perm_ref2int)


def _w3_permute(W3b):
    """Permute + scale the (W3;b3) columns for the on-device TP layout.

    Input rows [65, 2304] in reference order:
      w00 col i*32+o, w10 1024+i*32+o, w01 1536+i*16+o, w11 2048+i*16+o.

    Output layout, scales folded.  Blocks 00/10/11 use a k-interleaved
    order (i = k*8 + r) so the i-contraction runs as flat contiguous
    half-adds on DVE followed by an 8-wide window-level reduce:
      00 region       k*256 + o*8 + r (k<4,o<32,r<8): alpha*w00[k*8+r, o]
      10 region 1024+ k*256 + o*8 + r (k<2,o<32,r<8): a/sqrt3*w10[k*8+r,o]
      01 region 1536+ o*32 + i       (o<16,i<32):     alpha*w01[i,o]
      11 region 2048+ k*128 + o*8 + r (k<2,o<16,r<8): alpha*w11[k*8+r,o]
    """
    idx = np.empty(WNUM, np.int64)
    scl = np.empty(WNUM, np.float32)
    for k in range(4):
        for o in range(32):
            for r in range(8):
                idx[k * 256 + o * 8 + r] = (k * 8 + r) * 32 + o
                scl[k * 256 + o * 8 + r] = ALPHA
    for k in range(2):
        for o in range(32):
            for r in range(8):
                idx[1024 + k * 256 + o * 8 + r] = 1024 + (k * 8 + r) * 32 + o
                scl[1024 + k * 256 + o * 8 + r] = ALPHA * INV_SQRT3
    for o in range(16):
        for i in range(32):
            idx[1536 + o * 32 + i] = 1536 + i * 16 + o
            scl[1536 + o * 32 + i] = ALPHA
    for k in range(2):
        for o in range(16):
            for r in range(8):
                idx[2048 + k * 128 + o * 8 + r] = 2048 + (k * 8 + r) * 16 + o
                scl[2048 + k * 128 + o * 8 + r] = ALPHA
    return (W3b[:, idx] * scl[None, :]).astype(np.float32)


def _balance_nodes(dst):
    """Assign each core's local nodes to N_WIN windows (<=NPW nodes each),
    balancing per-window edge counts (greedy LPT).  Returns win_of, pos_of
    [N_CORES, NPC] and per-(core,window) edge counts."""
    core = dst // NPC
    dloc = dst - core * NPC
    win_of = np.zeros((N_CORES, NPC), np.int64)
    pos_of = np.zeros((N_CORES, NPC), np.int64)
    ecnt = np.zeros((N_CORES, N_WIN), np.int64)
    for c in range(N_CORES):
        deg = np.bincount(dloc[core == c], minlength=NPC)
        order = np.argsort(-deg, kind="stable")
        loads = np.zeros(N_WIN, np.int64)
        counts = np.zeros(N_WIN, np.int64)
        for n in order:
            open_w = np.nonzero(counts < NPW)[0]
            w = open_w[np.argmin(loads[open_w])]
            win_of[c, n] = w
            pos_of[c, n] = counts[w]
            counts[w] += 1
            loads[w] += deg[n]
        ecnt[c] = loads
    return win_of, pos_of, ecnt


def _shard(edge_index):
    src, dst = edge_index[0], edge_index[1]
    win_of, pos_of, ecnt = _balance_nodes(dst)
    tiles_per_win = [
        max(1, int(max((ecnt[c, w] + ET - 1) // ET for c in range(N_CORES))))
        for w in range(N_WIN)
    ]
    core = dst // NPC
    dloc = dst - core * NPC
    order = [[None] * N_WIN for _ in range(N_CORES)]
    for c in range(N_CORES):
        idx = np.nonzero(core == c)[0]
        w_of = win_of[c][dloc[idx]]
        s = np.argsort(w_of, kind="stable")
        idx = idx[s]
        w_of = w_of[s]
        bounds = np.searchsorted(w_of, np.arange(N_WIN + 1))
        for w in range(N_WIN):
            order[c][w] = idx[bounds[w]:bounds[w + 1]]
    return order, tiles_per_win, win_of, pos_of


def _build_core_inputs(inputs, order, tiles_per_win, win_of, pos_of, c, bf16):
    h = inputs["h"]
    edge_sh = inputs["edge_sh"]
    ef = inputs["edge_features"]
    src = inputs["edge_index"][0]
    dst = inputs["edge_index"][1]

    T = int(sum(tiles_per_win))
    E_pad = T * ET
    svx = np.zeros((E_pad, 96), np.float32)
    efp = np.zeros((E_pad, EDGE_DIM), np.float32)

    pos = 0
    for w in range(N_WIN):
        ids = order[c][w]
        n = len(ids)
        sl = slice(pos, pos + n)
        hs = h[src[ids]][:, _perm_ref2int]     # [n, 80] internal layout
        svx[sl, 0:80] = hs
        svx[sl, 80] = edge_sh[ids, 0]
        svx[sl, 81:84] = edge_sh[ids, 1:4]
        svx[sl, 84] = 1.0
        svx[sl, 85] = -pos_of[c][dst[ids] - c * NPC].astype(np.float32)
        efp[sl] = ef[ids]
        pos += tiles_per_win[w] * ET
    efT = np.ascontiguousarray(
        efp.reshape(T, ET, EDGE_DIM).transpose(0, 2, 1)
    )

    # node-side layouts follow the per-core window permutation
    hsl = h[c * NPC:(c + 1) * NPC][:, _perm_ref2int]   # [2500, 80] internal
    col = win_of[c] * WIN + pos_of[c]                  # node -> slab column
    hT_s = np.zeros((33, NPAD), np.float32)
    hT_s[0, col] = 1.0
    hT_s[1:, col] = hsl[:, :32].T
    hT_v = [np.zeros((16, NPAD), np.float32) for _ in range(3)]
    for cc in range(3):
        hT_v[cc][:, col] = hsl[:, 32 + cc * 16:32 + (cc + 1) * 16].T
    h_nm = np.zeros((WIN, N_WIN, 80), np.float32)      # node-major slab
    h_nm[pos_of[c], win_of[c]] = hsl

    return dict(
        svx=svx.reshape(T, ET, 96).astype(bf16),
        efT=efT.astype(bf16),
        hT_s=hT_s.astype(bf16),
        hT_v0=hT_v[0].astype(bf16),
        hT_v1=hT_v[1].astype(bf16),
        hT_v2=hT_v[2].astype(bf16),
        h_nm=h_nm,
    ), T


def _fused_bass(T, tiles_per_win):
    import concourse.bacc as bacc
    import concourse.mybir as mybir
    import concourse.tile as tile

    fp32 = mybir.dt.float32
    bf16 = mybir.dt.bfloat16
    Alu = mybir.AluOpType
    Act = mybir.ActivationFunctionType
    X = mybir.AxisListType.X

    nc = bacc.Bacc("TRN2", target_bir_lowering=False, debug=False,
                   num_devices=N_CORES)
    d_svx = nc.dram_tensor("svx", [T, ET, 96], bf16, kind="ExternalInput")
    d_efT = nc.dram_tensor("efT", [T, 32, ET], bf16, kind="ExternalInput")
    d_hTs = nc.dram_tensor("hT_s", [33, NPAD], bf16, kind="ExternalInput")
    d_hTv = [
        nc.dram_tensor(f"hT_v{i}", [16, NPAD], bf16, kind="ExternalInput")
        for i in range(3)
    ]
    d_hnm = nc.dram_tensor("h_nm", [WIN, N_WIN, 80], fp32,
                           kind="ExternalInput")
    d_W1 = nc.dram_tensor("W1", [32, 64], bf16, kind="ExternalInput")
    d_b1 = nc.dram_tensor("b1", [64, 1], fp32, kind="ExternalInput")
    d_W2 = nc.dram_tensor("W2", [64, 64], bf16, kind="ExternalInput")
    d_b2 = nc.dram_tensor("b2", [64, 1], fp32, kind="ExternalInput")
    d_W3 = nc.dram_tensor("W3", [65, WNUM], bf16, kind="ExternalInput")
    d_Wn = nc.dram_tensor("Wn", [33, 64], bf16, kind="ExternalInput")
    d_Wv = nc.dram_tensor("Wv", [16, 32], bf16, kind="ExternalInput")
    d_bnw = nc.dram_tensor("bnw", [1, 80], fp32, kind="ExternalInput")
    d_bnb = nc.dram_tensor("bnb", [1, 80], fp32, kind="ExternalInput")
    d_out = nc.dram_tensor("out", [WIN, N_WIN, 80], fp32,
                           kind="ExternalOutput")

    with tile.TileContext(nc) as tc, \
            nc.allow_low_precision(reason="bf16 TP well within 2e-2 tol"):
        with tc.tile_pool(name="singles", bufs=1) as singles:
            sW1 = singles.tile([32, 64], bf16)
            nc.sync.dma_start(out=sW1, in_=d_W1[:, :])
            sb1 = singles.tile([64, 1], fp32)
            nc.sync.dma_start(out=sb1, in_=d_b1[:, :])
            sW2 = singles.tile([64, 64], bf16)
            nc.sync.dma_start(out=sW2, in_=d_W2[:, :])
            sb2 = singles.tile([64, 1], fp32)
            nc.sync.dma_start(out=sb2, in_=d_b2[:, :])
            sW3 = singles.tile([65, WNUM], bf16)
            nc.sync.dma_start(out=sW3, in_=d_W3[:, :])
            sWn = singles.tile([33, 64], bf16)
            nc.sync.dma_start(out=sWn, in_=d_Wn[:, :])
            sWv = singles.tile([16, 32], bf16)
            nc.sync.dma_start(out=sWv, in_=d_Wv[:, :])
            sHs = singles.tile([33, NPAD], bf16)
            nc.sync.dma_start(out=sHs, in_=d_hTs[:, :])
            sHv = []
            for i in range(3):
                t = singles.tile([16, NPAD], bf16, tag=f"hv{i}")
                nc.sync.dma_start(out=t, in_=d_hTv[i][:, :])
                sHv.append(t)
            # h_nm is only read in the end phase; DMA issued there so the
            # 820KB transfer doesn't stall the first edge tiles' loads
            h_nm = singles.tile([WIN, N_WIN, 80], fp32)
            bnw = singles.tile([1, 80], fp32)
            nc.sync.dma_start(out=bnw, in_=d_bnw[:, :])
            bnb = singles.tile([1, 80], fp32)
            nc.sync.dma_start(out=bnb, in_=d_bnb[:, :])

            iotaI = singles.tile([ET, WIN], mybir.dt.int32)
            nc.gpsimd.iota(iotaI, [[1, WIN]], channel_multiplier=0)
            iotaB = singles.tile([ET, WIN], bf16)
            nc.vector.tensor_copy(iotaB, iotaI)
            ones_col = singles.tile([WIN, 1], bf16)
            nc.vector.memset(ones_col, 1.0)
            ones_row = singles.tile([1, WIN], fp32)
            nc.vector.memset(ones_row, 1.0)

            slab_st = singles.tile([WIN, N_WIN, 160], bf16)
            slab_g = singles.tile([WIN, N_WIN, 80], bf16)
            slab_out = singles.tile([WIN, N_WIN, 80], fp32)

            # manual double-buffer for x2a: row 64 is a constant ones row
            # (the b3 contraction row), written once — pools would force a
            # per-tile rewrite.
            x2a_bufs = []
            for i in range(2):
                xt = singles.tile([65, ET], bf16, tag=f"x2a{i}")
                nc.vector.memset(xt[64:65, :], 1.0)
                x2a_bufs.append(xt)

            # ---------------- edge + node phase (per window) ----------------
            # scatter payload layout (945 bf16 cols per edge):
            #   0:256    00-block partial sums, (o=32, r=8)
            #   256:512  10-block partial sums, (o=32, r=8)
            #   512:896  11-block partial sums, (c=3, o=16, r=8)
            #   896:944  p01*sh1, (c=3, o=16)
            #   944      valid (degree)
            with (
                tc.tile_pool(name="edma", bufs=6) as edma,
                tc.tile_pool(name="esb", bufs=3) as esb,
                tc.tile_pool(name="nsb", bufs=2) as nsb,
                tc.tile_pool(name="wpsA", bufs=1, space="PSUM") as wpsA,
                tc.tile_pool(name="wps01", bufs=1, space="PSUM") as wps01,
                tc.tile_pool(name="wpsB", bufs=1, space="PSUM") as wpsB,
                tc.tile_pool(name="mmout", bufs=1, space="PSUM") as mmout,
                tc.tile_pool(name="sps", bufs=1, space="PSUM") as sps,
            ):
                # self-interaction + gate pre-acts for ALL windows up front:
                # they depend only on constant slabs, and hoisting them off
                # the per-window path removes the mmout-bank serialization
                # at every window boundary.  Layout per window:
                # [si_s(32) | gate_s(32) | 3 x (si_v(16) | gate_v(16))]
                slab_nv = singles.tile([WIN, N_WIN, 160], fp32)
                for w in range(N_WIN):
                    nmo = mmout.tile([128, 160], fp32, tag="mm")
                    nc.tensor.matmul(nmo[:, 0:64],
                                     sHs[:, w * WIN:(w + 1) * WIN], sWn,
                                     start=True, stop=True)
                    for cc in range(3):
                        nc.tensor.matmul(
                            nmo[:, 64 + cc * 32:96 + cc * 32],
                            sHv[cc][:, w * WIN:(w + 1) * WIN], sWv,
                            start=True, stop=True)
                    nc.scalar.activation(slab_nv[:, w, :], nmo, Act.Copy)

                t_idx = 0
                for w in range(N_WIN):
                    ps_sum = sps.tile([WIN, 945], fp32, tag="scat")
                    jlast = tiles_per_win[w] - 1
                    for j in range(tiles_per_win[w]):
                        t = t_idx
                        t_idx += 1
                        sv = edma.tile([ET, 96], bf16, tag="svx")
                        nc.sync.dma_start(out=sv, in_=d_svx[t, :, :])
                        ef_t = edma.tile([32, ET], bf16, tag="ef")
                        nc.sync.dma_start(out=ef_t, in_=d_efT[t, :, :])

                        # --- edge MLP (feature-major, bf16) ---
                        mo1 = mmout.tile([128, 160], fp32, tag="mm")
                        nc.tensor.matmul(mo1[0:64, 0:128], sW1, ef_t,
                                         start=True, stop=True)
                        x1 = esb.tile([64, ET], bf16, tag="x1")
                        nc.scalar.activation(x1, mo1[0:64, 0:128], Act.Silu,
                                             bias=sb1)
                        mo2 = mmout.tile([128, 160], fp32, tag="mm")
                        nc.tensor.matmul(mo2[0:64, 0:128], sW2, x1,
                                         start=True, stop=True)
                        x2a = x2a_bufs[t % 2]
                        nc.scalar.activation(x2a[0:64, :], mo2[0:64, 0:128],
                                             Act.Silu, bias=sb2)

                        # --- mm3: per-edge TP weights, 3 PSUM regions ---
                        psA = wpsA.tile([ET, 1536], fp32, tag="A")
                        for c0 in (0, 512, 1024):
                            nc.tensor.matmul(psA[:, c0:c0 + 512], x2a,
                                             sW3[:, c0:c0 + 512],
                                             start=True, stop=True)
                        ps01 = wps01.tile([ET, 512], fp32, tag="o1")
                        nc.tensor.matmul(ps01, x2a, sW3[:, 1536:2048],
                                         start=True, stop=True)
                        psB = wpsB.tile([ET, 256], fp32, tag="B")
                        nc.tensor.matmul(psB, x2a, sW3[:, 2048:2304],
                                         start=True, stop=True)

                        # --- per-edge features ---
                        # fp32 copies of the per-edge scalars (ts needs f32)
                        aux32 = esb.tile([ET, 6], fp32, tag="aux32")
                        nc.gpsimd.tensor_copy(aux32, sv[:, 80:86])
                        # fAV = [se*sh0 (32) | dv (16) | vec*sh0 (48)]
                        # scale-by-partition-scalar runs on ACT (idle)
                        fAV = esb.tile([ET, 96], bf16, tag="fAV")
                        nc.scalar.activation(
                            fAV[:, 0:32], sv[:, 0:32], Act.Copy,
                            scale=aux32[:, 0:1])
                        nc.scalar.activation(
                            fAV[:, 48:96], sv[:, 32:80], Act.Copy,
                            scale=aux32[:, 0:1])
                        t3 = esb.tile([ET, 48], bf16, tag="t3")
                        nc.gpsimd.tensor_tensor(
                            out=t3, in0=sv[:, 32:80],
                            in1=sv[:, 81:84].unsqueeze(2).broadcast_to(
                                (ET, 3, 16)),
                            op=Alu.mult)
                        nc.vector.tensor_reduce(
                            out=fAV[:, 32:48],
                            in_=t3.rearrange("p (c i) -> p i c", c=3),
                            axis=X, op=Alu.add)

                        # --- TP products: DVE straight from PSUM ---
                        prod00 = esb.tile([ET, 1024], bf16, tag="prod00")
                        nc.vector.tensor_tensor(
                            out=prod00.rearrange("p (k o r) -> p k o r",
                                                 k=4, o=32),
                            in0=psA[:, 0:1024].rearrange(
                                "p (k o r) -> p k o r", k=4, o=32),
                            in1=fAV[:, 0:32]
                                .rearrange("p (k r) -> p k r", k=4)
                                .unsqueeze(2).broadcast_to((ET, 4, 32, 8)),
                            op=Alu.mult)
                        prod10 = esb.tile([ET, 512], bf16, tag="prod10")
                        nc.vector.tensor_tensor(
                            out=prod10.rearrange("p (k o r) -> p k o r",
                                                 k=2, o=32),
                            in0=psA[:, 1024:1536].rearrange(
                                "p (k o r) -> p k o r", k=2, o=32),
                            in1=fAV[:, 32:48]
                                .rearrange("p (k r) -> p k r", k=2)
                                .unsqueeze(2).broadcast_to((ET, 2, 32, 8)),
                            op=Alu.mult)
                        fVv = fAV[:, 48:96].rearrange("p (c i) -> p c i",
                                                      c=3)
                        prodBk = []
                        for k in range(2):
                            pk = esb.tile([ET, 384], bf16, tag=f"prodB{k}")
                            nc.vector.tensor_tensor(
                                out=pk.rearrange("p (c o r) -> p c o r",
                                                 c=3, o=16),
                                in0=psB[:, k * 128:(k + 1) * 128]
                                    .rearrange("p (o r) -> p o r", o=16)
                                    .unsqueeze(1)
                                    .broadcast_to((ET, 3, 16, 8)),
                                in1=fVv[:, :, k * 8:(k + 1) * 8]
                                    .unsqueeze(2)
                                    .broadcast_to((ET, 3, 16, 8)),
                                op=Alu.mult)
                            prodBk.append(pk)
                        w01 = esb.tile([ET, 512], bf16, tag="w01")
                        nc.scalar.activation(w01, ps01, Act.Copy)
                        prod01 = esb.tile([ET, 512], bf16, tag="prod01")
                        nc.gpsimd.tensor_tensor(
                            out=prod01.rearrange("p (o i) -> p o i", o=16),
                            in0=w01.rearrange("p (o i) -> p o i", o=16),
                            in1=sv[:, 0:32].unsqueeze(1).broadcast_to(
                                (ET, 16, 32)),
                            op=Alu.mult)

                        # --- fold k-halves: flat contiguous adds ---
                        msgw = esb.tile([ET, 945], bf16, tag="msgw")
                        t00 = esb.tile([ET, 512], bf16, tag="t00")
                        nc.vector.tensor_tensor(
                            out=t00, in0=prod00[:, 0:512],
                            in1=prod00[:, 512:1024], op=Alu.add)
                        nc.vector.tensor_tensor(
                            out=msgw[:, 0:256], in0=t00[:, 0:256],
                            in1=t00[:, 256:512], op=Alu.add)
                        nc.gpsimd.tensor_tensor(
                            out=msgw[:, 256:512], in0=prod10[:, 0:256],
                            in1=prod10[:, 256:512], op=Alu.add)
                        nc.gpsimd.tensor_tensor(
                            out=msgw[:, 512:896], in0=prodBk[0],
                            in1=prodBk[1], op=Alu.add)

                        # --- 01-block: full reduce + sh1 outer product ---
                        p01 = esb.tile([ET, 16], bf16, tag="p01")
                        nc.vector.tensor_reduce(
                            out=p01,
                            in_=prod01.rearrange("p (o i) -> p o i", o=16),
                            axis=X, op=Alu.add)
                        nc.gpsimd.tensor_tensor(
                            out=msgw[:, 896:944].rearrange(
                                "p (c o) -> p c o", c=3),
                            in0=p01.unsqueeze(1).broadcast_to((ET, 3, 16)),
                            in1=sv[:, 81:84].unsqueeze(2).broadcast_to(
                                (ET, 3, 16)),
                            op=Alu.mult)
                        nc.gpsimd.tensor_copy(msgw[:, 944:945], sv[:, 84:85])

                        # --- one-hot scatter matmul (moving dim <= 512) ---
                        # one-hot on ACT: relu(1 - |iota - dstw|); svx col 85
                        # holds -dstw so it can ride the activation bias
                        absd = esb.tile([ET, WIN], bf16, tag="absd")
                        nc.scalar.activation(absd, iotaB, Act.Abs,
                                             bias=aux32[:, 5:6])
                        S = esb.tile([ET, WIN], bf16, tag="S")
                        nc.scalar.activation(S, absd, Act.Relu,
                                             scale=-1.0, bias=1.0)
                        for c0, c1 in ((0, 512), (512, 945)):
                            nc.tensor.matmul(
                                ps_sum[:, c0:c1], S, msgw[:, c0:c1],
                                start=(j == 0), stop=(j == jlast),
                                skip_group_check=True)

                    # ---------------- node phase for window w ----------------
                    # window-level reduce of the scattered 8-wide partials
                    tw = nsb.tile([WIN, 112], fp32, tag="tw")
                    nc.vector.tensor_reduce(
                        out=tw,
                        in_=ps_sum[:, 0:896].rearrange(
                            "p (g r) -> p g r", g=112),
                        axis=X, op=Alu.add)
                    summed = nsb.tile([WIN, 80], fp32, tag="summed")
                    nc.vector.tensor_tensor(
                        out=summed[:, 0:32], in0=tw[:, 0:32],
                        in1=tw[:, 32:64], op=Alu.add)
                    nc.vector.tensor_tensor(
                        out=summed[:, 32:80], in0=tw[:, 64:112],
                        in1=ps_sum[:, 896:944], op=Alu.add)
                    degc = nsb.tile([WIN, 1], fp32, tag="degc")
                    nc.vector.tensor_scalar(
                        degc, ps_sum[:, 944:945], 1.0, None, op0=Alu.max)
                    rdeg = nsb.tile([WIN, 1], fp32, tag="rdeg")
                    nc.vector.reciprocal(rdeg, degc)
                    agg = nsb.tile([WIN, 80], fp32, tag="agg")
                    nc.vector.tensor_scalar(
                        agg, summed, rdeg, None, op0=Alu.mult)

                    # upd -> slab_st[:, w, 0:80]; sq -> [:, w, 80:160]
                    # (self-interaction comes from the SBUF slab, so these
                    # run on Pool; sigmoids are batched in the end phase)
                    nv_v = slab_nv[:, w, 64:160].rearrange(
                        "p (c k) -> p c k", c=3)
                    nc.gpsimd.tensor_tensor(
                        out=slab_st[:, w, 0:32], in0=agg[:, 0:32],
                        in1=slab_nv[:, w, 0:32], op=Alu.add)
                    nc.gpsimd.tensor_tensor(
                        out=slab_st[:, w, 32:80].rearrange(
                            "p (c i) -> p c i", c=3),
                        in0=agg[:, 32:80].rearrange("p (c i) -> p c i", c=3),
                        in1=nv_v[:, :, 0:16], op=Alu.add)
                    nc.gpsimd.tensor_tensor(
                        out=slab_st[:, w, 80:160],
                        in0=slab_st[:, w, 0:80],
                        in1=slab_st[:, w, 0:80], op=Alu.mult)

            # ---------------- end phase: stats, allreduce, BN, update -------
            with (
                tc.tile_pool(name="eps", bufs=1, space="PSUM") as eps_p,
                tc.tile_pool(name="fsb", bufs=1) as fsb,
                tc.tile_pool(name="dram", bufs=2, space="DRAM") as dram,
            ):
                nc.sync.dma_start(out=h_nm, in_=d_hnm[:, :, :])
                # all gate sigmoids in two batched instrs (one table load)
                nc.scalar.activation(
                    slab_g[:, :, 0:32], slab_nv[:, :, 32:64], Act.Sigmoid)
                nc.scalar.activation(
                    slab_g[:, :, 32:80].rearrange(
                        "p w (c i) -> p w c i", c=3),
                    slab_nv[:, :, 64:160].rearrange(
                        "p w (c k) -> p w c k", c=3)[:, :, :, 16:32],
                    Act.Sigmoid)
                ps_st = eps_p.tile([1, 160], fp32, tag="st")
                for w in range(N_WIN):
                    nc.tensor.matmul(
                        ps_st, ones_col, slab_st[:, w, :],
                        start=(w == 0), stop=(w == N_WIN - 1),
                        skip_group_check=True)
                st_sb = fsb.tile([1, 160], fp32, tag="stsb")
                nc.scalar.activation(st_sb, ps_st, Act.Copy)

                ib = dram.tile([1, 160], fp32, tag="ib")
                ob = dram.tile([1, 160], fp32, tag="ob")
                nc.gpsimd.dma_start(ib[:], st_sb[:])
                nc.gpsimd.collective_compute(
                    "AllReduce", mybir.AluOpType.add,
                    replica_groups=[list(range(N_CORES))],
                    ins=[ib.opt()], outs=[ob.opt()])
                st_r = fsb.tile([1, 160], fp32, tag="str")
                nc.gpsimd.dma_start(st_r[:], ob[:])

                inv_n = 1.0 / float(N_NODES)
                meanb = fsb.tile([1, 80], fp32, tag="meanb")
                nc.vector.tensor_scalar(
                    meanb, st_r[:, 0:80], inv_n, None, op0=Alu.mult)
                nc.vector.memset(meanb[:, 32:80], 0.0)
                ex2 = fsb.tile([1, 80], fp32, tag="ex2")
                nc.vector.tensor_scalar(
                    ex2, st_r[:, 80:160], inv_n, None, op0=Alu.mult)
                m2 = fsb.tile([1, 80], fp32, tag="m2")
                nc.vector.tensor_tensor(out=m2, in0=meanb, in1=meanb,
                                        op=Alu.mult)
                exm = fsb.tile([1, 80], fp32, tag="exm")
                nc.vector.tensor_tensor(out=exm, in0=ex2, in1=m2,
                                        op=Alu.subtract)
                vn = fsb.tile([1, 16], fp32, tag="vn")
                nc.vector.tensor_reduce(
                    out=vn,
                    in_=exm[:, 32:80].rearrange("p (c i) -> p i c", c=3),
                    axis=X, op=Alu.add)
                varb = fsb.tile([1, 80], fp32, tag="varb")
                nc.vector.tensor_scalar(
                    varb[:, 0:32], exm[:, 0:32], 1.0, float(EPS),
                    op0=Alu.mult, op1=Alu.add)
                nc.vector.tensor_scalar(
                    varb[:, 32:80].rearrange("p (c i) -> p c i", c=3),
                    vn.unsqueeze(1).broadcast_to((1, 3, 16)),
                    1.0 / 3.0, float(EPS), op0=Alu.mult, op1=Alu.add)
                rec = fsb.tile([1, 80], fp32, tag="rec")
                nc.vector.reciprocal(rec, varb)
                rstd = fsb.tile([1, 80], fp32, tag="rstd")
                nc.scalar.activation(rstd, rec, Act.Sqrt)
                scsh = fsb.tile([1, 160], fp32, tag="scsh")
                nc.vector.tensor_tensor(
                    out=scsh[:, 0:80], in0=rstd, in1=bnw, op=Alu.mult)
                msc = fsb.tile([1, 80], fp32, tag="msc")
                nc.vector.tensor_tensor(
                    out=msc, in0=meanb, in1=scsh[:, 0:80], op=Alu.mult)
                nc.vector.tensor_tensor(
                    out=scsh[:, 80:160], in0=bnb, in1=msc, op=Alu.subtract)

                ps_b = eps_p.tile([128, 160], fp32, tag="bc")
                nc.tensor.matmul(ps_b, ones_row, scsh, start=True, stop=True)
                scshB = fsb.tile([128, 160], fp32, tag="scshB")
                nc.scalar.activation(scshB, ps_b, Act.Copy)

                # batched gated residual update over the whole node slab:
                # out = (upd*sc + sh) * g + h, broadcasting sc/sh per window
                scB = scshB[:, 0:80].unsqueeze(1).broadcast_to(
                    (WIN, N_WIN, 80))
                shB = scshB[:, 80:160].unsqueeze(1).broadcast_to(
                    (WIN, N_WIN, 80))
                t1 = fsb.tile([WIN, N_WIN, 80], fp32, tag="t1")
                nc.vector.tensor_tensor(
                    out=t1, in0=slab_st[:, :, 0:80], in1=scB, op=Alu.mult)
                t2 = fsb.tile([WIN, N_WIN, 80], fp32, tag="t2")
                nc.gpsimd.tensor_tensor(
                    out=t2, in0=t1, in1=shB, op=Alu.add)
                nc.vector.tensor_tensor(
                    out=t1, in0=t2, in1=slab_g, op=Alu.mult)
                nc.gpsimd.tensor_tensor(
                    out=slab_out, in0=t1, in1=h_nm, op=Alu.add)
                nc.sync.dma_start(out=d_out[:, :, :], in_=slab_out)
    nc.compile()
    return nc


def kernel(**inputs):
    import os
    from concourse.bass_utils import run_bass_kernel_spmd
    import ml_dtypes

    bf16 = ml_dtypes.bfloat16
    trace = os.environ.get("KERNEL_TRACE", "0") == "1"
    inputs = {k: np.asarray(v) for k, v in inputs.items()}
    edge_index = inputs["edge_index"].astype(np.int64)
    inputs["edge_index"] = edge_index
    for k in list(inputs):
        if inputs[k].dtype == np.float64:
            inputs[k] = inputs[k].astype(np.float32)

    order, tiles_per_win, win_of, pos_of = _shard(edge_index)

    # shared weights (host-side dtype conversion / packing)
    W3b = np.vstack([
        inputs["W3"].astype(np.float32),
        inputs["b3"].astype(np.float32).reshape(1, WNUM),
    ])
    W3p = _w3_permute(W3b).astype(bf16)
    Wn = np.zeros((33, 64), np.float32)
    Wn[0, 0:32] = inputs["bs_s"]
    Wn[1:, 0:32] = inputs["ws_s"]
    Wn[0, 32:64] = inputs["bg_s"]
    Wn[1:, 32:64] = inputs["wg_s"]
    Wv = np.zeros((16, 32), np.float32)
    Wv[:, 0:16] = inputs["ws_v"]
    Wv[:, 16:32] = inputs["wg_v"]
    bnw_row = np.zeros((1, 80), np.float32)
    bnw_row[0, 0:32] = inputs["bn_ws"]
    for cc in range(3):
        bnw_row[0, 32 + cc * 16:48 + cc * 16] = inputs["bn_wv"]
    bnb_row = np.zeros((1, 80), np.float32)
    bnb_row[0, 0:32] = inputs["bn_bs"]

    core_maps = []
    T = None
    for c in range(N_CORES):
        m, T = _build_core_inputs(
            inputs, order, tiles_per_win, win_of, pos_of, c, bf16)
        m["W1"] = inputs["W1"].astype(bf16)
        m["b1"] = inputs["b1"].astype(np.float32).reshape(64, 1)
        m["W2"] = inputs["W2"].astype(bf16)
        m["b2"] = inputs["b2"].astype(np.float32).reshape(64, 1)
        m["W3"] = W3p
        m["Wn"] = Wn.astype(bf16)
        m["Wv"] = Wv.astype(bf16)
        m["bnw"] = bnw_row
        m["bnb"] = bnb_row
        core_maps.append(m)

    nc1 = _fused_bass(T, tiles_per_win)
    r1 = run_bass_kernel_spmd(
        nc1, core_maps, core_ids=list(range(N_CORES)), trace=trace)
    global _last_exec_ns
    _last_exec_ns = r1.exec_time_ns
    _last_rr.clear()
    _last_rr.append(r1)

    out = np.zeros((N_NODES, 80), np.float32)
    for c in range(N_CORES):
        slab = r1.results[c]["out"]          # [WIN, N_WIN, 80]
        blk = slab[pos_of[c], win_of[c]]     # [2500, 80] internal layout
        out[c * NPC:(c + 1) * NPC] = blk[:, _perm_int2ref]
    return out


if __name__ == "__main__":
    import reference

    inp = reference.setup_inputs()
    inp = {k: np.asarray(v) for k, v in inp.items()}
    expected = np.asarray(reference.reference(**inp))
    actual = kernel(**inp)
    err = np.abs(actual - expected)
    rel = np.linalg.norm(actual - expected) / np.linalg.norm(expected)
    print("max abs err:", err.max(), "rel:", rel)


# revision 50
# speedup vs baseline: 8.5361x; 1.0075x over previous
"""Trainium2 Bass kernel for nn_EquivariantBlock (gnn_message_passing).

Single fused kernel, nodes partitioned across 8 cores (2500 each).
Host does sharding/gather/layout only; all FLOPs run on device.

Per core: edges grouped by destination window (128-node windows, node->window
assignment load-balanced via LPT so every window needs ~the same tile count).
Edge phase per 128-edge tile: bf16 edge MLP on PE -> per-edge TP weights in
PSUM (never touch HBM; W3 columns pre-permuted + CG/alpha scales folded on
host) -> ACT evacuates weights to SBUF bf16 -> DVE forms per-edge products
(plain tensor_tensor, 2x bf16 mode) -> Pool engine reduces -> one-hot scatter
matmul accumulates per-window sums in PSUM.  Node phase interleaved per
window: degree-mean, self-interaction + gate matmuls (bf16), batch-stat slab.
End phase: stats matmul -> 640B AllReduce across the 8 cores -> BN scale/shift
vectors on-device -> broadcast via matmul -> gated residual update -> one DMA.
"""

import numpy as np

MUL0, MUL1 = 32, 16
EDGE_DIM, HID = 32, 64
WNUM = 2304
N_NODES, N_EDGES = 20000, 100000
EPS = 1e-5
ALPHA = 1.0 / np.sqrt(48.0)
INV_SQRT3 = 1.0 / np.sqrt(3.0)

N_CORES = 8
NPC = N_NODES // N_CORES          # nodes per core = 2500
N_WIN = 20                        # windows per core
NPW = NPC // N_WIN                # nodes per window = 125 (< 128)
WIN = 128                         # window slot size (partition dim)
NPAD = N_WIN * WIN                # 2560
ET = 128                          # edges per tile

_last_exec_ns = None
_last_rr = []

# column permutation: reference h layout [s(32), v (i-major: i*3+c)]
# internal layout  [s(32), v (c-major: c*16+i)]
_perm_ref2int = np.concatenate(
    [np.arange(32)] + [32 + np.arange(16) * 3 + c for c in range(3)]
).astype(np.int64)
_perm_int2ref = np.argsort(_perm_ref2int)


def _w3_permute(W3b):
    """Permute + scale the (W3;b3) columns for the on-device TP layout.

    Input rows [65, 2304] in reference order:
      w00 col i*32+o, w10 1024+i*32+o, w01 1536+i*16+o, w11 2048+i*16+o.

    Output layout, scales folded.  Blocks 00/10/11 use a k-interleaved
    order (i = k*8 + r) so the i-contraction runs as flat contiguous
    half-adds on DVE followed by an 8-wide window-level reduce:
      00 region       k*256 + o*8 + r (k<4,o<32,r<8): alpha*w00[k*8+r, o]
      10 region 1024+ k*256 + o*8 + r (k<2,o<32,r<8): a/sqrt3*w10[k*8+r,o]
      01 region 1536+ o*32 + i       (o<16,i<32):     alpha*w01[i,o]
      11 region 2048+ k*128 + o*8 + r (k<2,o<16,r<8): alpha*w11[k*8+r,o]
    """
    idx = np.empty(WNUM, np.int64)
    scl = np.empty(WNUM, np.float32)
    for k in range(4):
        for o in range(32):
            for r in range(8):
                idx[k * 256 + o * 8 + r] = (k * 8 + r) * 32 + o
                scl[k * 256 + o * 8 + r] = ALPHA
    for k in range(2):
        for o in range(32):
            for r in range(8):
                idx[1024 + k * 256 + o * 8 + r] = 1024 + (k * 8 + r) * 32 + o
                scl[1024 + k * 256 + o * 8 + r] = ALPHA * INV_SQRT3
    for o in range(16):
        for i in range(32):
            idx[1536 + o * 32 + i] = 1536 + i * 16 + o
            scl[1536 + o * 32 + i] = ALPHA
    for k in range(2):
        for o in range(16):
            for r in range(8):
                idx[2048 + k * 128 + o * 8 + r] = 2048 + (k * 8 + r) * 16 + o
                scl[2048 + k * 128 + o * 8 + r] = ALPHA
    return (W3b[:, idx] * scl[None, :]).astype(np.float32)


def _balance_nodes(dst):
    """Assign each core's local nodes to N_WIN windows (<=NPW nodes each),
    balancing per-window edge counts (greedy LPT).  Returns win_of, pos_of
    [N_CORES, NPC] and per-(core,window) edge counts."""
    core = dst // NPC
    dloc = dst - core * NPC
    win_of = np.zeros((N_CORES, NPC), np.int64)
    pos_of = np.zeros((N_CORES, NPC), np.int64)
    ecnt = np.zeros((N_CORES, N_WIN), np.int64)
    for c in range(N_CORES):
        deg = np.bincount(dloc[core == c], minlength=NPC)
        order = np.argsort(-deg, kind="stable")
        loads = np.zeros(N_WIN, np.int64)
        counts = np.zeros(N_WIN, np.int64)
        for n in order:
            open_w = np.nonzero(counts < NPW)[0]
            w = open_w[np.argmin(loads[open_w])]
            win_of[c, n] = w
            pos_of[c, n] = counts[w]
            counts[w] += 1
            loads[w] += deg[n]
        ecnt[c] = loads
    return win_of, pos_of, ecnt


def _shard(edge_index):
    src, dst = edge_index[0], edge_index[1]
    win_of, pos_of, ecnt = _balance_nodes(dst)
    tiles_per_win = [
        max(1, int(max((ecnt[c, w] + ET - 1) // ET for c in range(N_CORES))))
        for w in range(N_WIN)
    ]
    core = dst // NPC
    dloc = dst - core * NPC
    order = [[None] * N_WIN for _ in range(N_CORES)]
    for c in range(N_CORES):
        idx = np.nonzero(core == c)[0]
        w_of = win_of[c][dloc[idx]]
        s = np.argsort(w_of, kind="stable")
        idx = idx[s]
        w_of = w_of[s]
        bounds = np.searchsorted(w_of, np.arange(N_WIN + 1))
        for w in range(N_WIN):
            order[c][w] = idx[bounds[w]:bounds[w + 1]]
    return order, tiles_per_win, win_of, pos_of


def _build_core_inputs(inputs, order, tiles_per_win, win_of, pos_of, c, bf16):
    h = inputs["h"]
    edge_sh = inputs["edge_sh"]
    ef = inputs["edge_features"]
    src = inputs["edge_index"][0]
    dst = inputs["edge_index"][1]

    T = int(sum(tiles_per_win))
    E_pad = T * ET
    svx = np.zeros((E_pad, 96), np.float32)
    efp = np.zeros((E_pad, EDGE_DIM), np.float32)

    pos = 0
    for w in range(N_WIN):
        ids = order[c][w]
        n = len(ids)
        sl = slice(pos, pos + n)
        hs = h[src[ids]][:, _perm_ref2int]     # [n, 80] internal layout
        svx[sl, 0:80] = hs
        svx[sl, 80] = edge_sh[ids, 0]
        svx[sl, 81:84] = edge_sh[ids, 1:4]
        svx[sl, 84] = 1.0
        svx[sl, 85] = -pos_of[c][dst[ids] - c * NPC].astype(np.float32)
        efp[sl] = ef[ids]
        pos += tiles_per_win[w] * ET
    efT = np.ascontiguousarray(
        efp.reshape(T, ET, EDGE_DIM).transpose(0, 2, 1)
    )

    # node-side layouts follow the per-core window permutation
    hsl = h[c * NPC:(c + 1) * NPC][:, _perm_ref2int]   # [2500, 80] internal
    col = win_of[c] * WIN + pos_of[c]                  # node -> slab column
    hT_all = np.zeros((81, NPAD), np.float32)
    hT_all[0, col] = 1.0
    hT_all[1:33, col] = hsl[:, :32].T
    for cc in range(3):
        hT_all[33 + cc * 16:49 + cc * 16, col] = \
            hsl[:, 32 + cc * 16:32 + (cc + 1) * 16].T
    h_nm = np.zeros((WIN, N_WIN, 80), np.float32)      # node-major slab
    h_nm[pos_of[c], win_of[c]] = hsl

    return dict(
        svx=svx.reshape(T, ET, 96).astype(bf16),
        efT=efT.astype(bf16),
        hT_all=hT_all.astype(bf16),
        h_nm=h_nm,
    ), T


def _fused_bass(T, tiles_per_win):
    import concourse.bacc as bacc
    import concourse.mybir as mybir
    import concourse.tile as tile

    fp32 = mybir.dt.float32
    bf16 = mybir.dt.bfloat16
    Alu = mybir.AluOpType
    Act = mybir.ActivationFunctionType
    X = mybir.AxisListType.X

    nc = bacc.Bacc("TRN2", target_bir_lowering=False, debug=False,
                   num_devices=N_CORES)
    d_svx = nc.dram_tensor("svx", [T, ET, 96], bf16, kind="ExternalInput")
    d_efT = nc.dram_tensor("efT", [T, 32, ET], bf16, kind="ExternalInput")
    d_hall = nc.dram_tensor("hT_all", [81, NPAD], bf16,
                            kind="ExternalInput")
    d_hnm = nc.dram_tensor("h_nm", [WIN, N_WIN, 80], fp32,
                           kind="ExternalInput")
    d_W1 = nc.dram_tensor("W1", [32, 64], bf16, kind="ExternalInput")
    d_b1 = nc.dram_tensor("b1", [64, 1], fp32, kind="ExternalInput")
    d_W2 = nc.dram_tensor("W2", [64, 64], bf16, kind="ExternalInput")
    d_b2 = nc.dram_tensor("b2", [64, 1], fp32, kind="ExternalInput")
    d_W3 = nc.dram_tensor("W3", [65, WNUM], bf16, kind="ExternalInput")
    d_wall = nc.dram_tensor("W_all", [81, 160], bf16, kind="ExternalInput")
    d_bnw = nc.dram_tensor("bnw", [1, 80], fp32, kind="ExternalInput")
    d_bnb = nc.dram_tensor("bnb", [1, 80], fp32, kind="ExternalInput")
    d_out = nc.dram_tensor("out", [WIN, N_WIN, 80], fp32,
                           kind="ExternalOutput")

    with tile.TileContext(nc) as tc, \
            nc.allow_low_precision(reason="bf16 TP well within 2e-2 tol"):
        with tc.tile_pool(name="singles", bufs=1) as singles:
            sW1 = singles.tile([32, 64], bf16)
            nc.sync.dma_start(out=sW1, in_=d_W1[:, :])
            sb1 = singles.tile([64, 1], fp32)
            nc.sync.dma_start(out=sb1, in_=d_b1[:, :])
            sW2 = singles.tile([64, 64], bf16)
            nc.sync.dma_start(out=sW2, in_=d_W2[:, :])
            sb2 = singles.tile([64, 1], fp32)
            nc.sync.dma_start(out=sb2, in_=d_b2[:, :])
            sW3 = singles.tile([65, WNUM], bf16)
            nc.sync.dma_start(out=sW3, in_=d_W3[:, :])
            sWall = singles.tile([81, 160], bf16)
            nc.sync.dma_start(out=sWall, in_=d_wall[:, :])
            sHall = singles.tile([81, NPAD], bf16)
            nc.sync.dma_start(out=sHall, in_=d_hall[:, :])
            # h_nm is only read in the end phase; DMA issued there so the
            # 820KB transfer doesn't stall the first edge tiles' loads
            h_nm = singles.tile([WIN, N_WIN, 80], fp32)
            bnw = singles.tile([1, 80], fp32)
            nc.sync.dma_start(out=bnw, in_=d_bnw[:, :])
            bnb = singles.tile([1, 80], fp32)
            nc.sync.dma_start(out=bnb, in_=d_bnb[:, :])

            iotaI = singles.tile([ET, WIN], mybir.dt.int32)
            nc.gpsimd.iota(iotaI, [[1, WIN]], channel_multiplier=0)
            iotaB = singles.tile([ET, WIN], bf16)
            nc.vector.tensor_copy(iotaB, iotaI)
            ones_col = singles.tile([WIN, 1], bf16)
            nc.vector.memset(ones_col, 1.0)
            ones_row = singles.tile([1, WIN], fp32)
            nc.vector.memset(ones_row, 1.0)

            slab_st = singles.tile([WIN, N_WIN, 160], bf16)
            slab_g = singles.tile([WIN, N_WIN, 80], bf16)
            slab_out = singles.tile([WIN, N_WIN, 80], fp32)

            # manual double-buffer for x2a: row 64 is a constant ones row
            # (the b3 contraction row), written once — pools would force a
            # per-tile rewrite.
            x2a_bufs = []
            for i in range(2):
                xt = singles.tile([65, ET], bf16, tag=f"x2a{i}")
                nc.vector.memset(xt[64:65, :], 1.0)
                x2a_bufs.append(xt)

            # ---------------- edge + node phase (per window) ----------------
            # scatter payload layout (945 bf16 cols per edge):
            #   0:256    00-block partial sums, (o=32, r=8)
            #   256:512  10-block partial sums, (o=32, r=8)
            #   512:896  11-block partial sums, (c=3, o=16, r=8)
            #   896:944  p01*sh1, (c=3, o=16)
            #   944      valid (degree)
            with (
                tc.tile_pool(name="edma", bufs=6) as edma,
                tc.tile_pool(name="esb", bufs=3) as esb,
                tc.tile_pool(name="nsb", bufs=2) as nsb,
                tc.tile_pool(name="wpsA", bufs=1, space="PSUM") as wpsA,
                tc.tile_pool(name="wps01", bufs=1, space="PSUM") as wps01,
                tc.tile_pool(name="wpsB", bufs=1, space="PSUM") as wpsB,
                tc.tile_pool(name="mmout", bufs=1, space="PSUM") as mmout,
                tc.tile_pool(name="sps", bufs=1, space="PSUM") as sps,
            ):
                # self-interaction + gate pre-acts for ALL windows up front:
                # they depend only on constant slabs, and hoisting them off
                # the per-window path removes the mmout-bank serialization
                # at every window boundary.  Layout per window:
                # [si_s(32) | gate_s(32) | 3 x (si_v(16) | gate_v(16))]
                slab_nv = singles.tile([WIN, N_WIN, 160], fp32)
                for w in range(N_WIN):
                    nmo = mmout.tile([128, 160], fp32, tag="mm")
                    nc.tensor.matmul(nmo,
                                     sHall[:, w * WIN:(w + 1) * WIN], sWall,
                                     start=True, stop=True)
                    nc.scalar.activation(slab_nv[:, w, :], nmo, Act.Copy)

                t_idx = 0
                for w in range(N_WIN):
                    ps_sum = sps.tile([WIN, 945], fp32, tag="scat")
                    jlast = tiles_per_win[w] - 1
                    for j in range(tiles_per_win[w]):
                        t = t_idx
                        t_idx += 1
                        sv = edma.tile([ET, 96], bf16, tag="svx")
                        nc.sync.dma_start(out=sv, in_=d_svx[t, :, :])
                        ef_t = edma.tile([32, ET], bf16, tag="ef")
                        nc.sync.dma_start(out=ef_t, in_=d_efT[t, :, :])

                        # --- edge MLP (feature-major, bf16) ---
                        mo1 = mmout.tile([128, 160], fp32, tag="mm")
                        nc.tensor.matmul(mo1[0:64, 0:128], sW1, ef_t,
                                         start=True, stop=True)
                        x1 = esb.tile([64, ET], bf16, tag="x1")
                        nc.scalar.activation(x1, mo1[0:64, 0:128], Act.Silu,
                                             bias=sb1)
                        mo2 = mmout.tile([128, 160], fp32, tag="mm")
                        nc.tensor.matmul(mo2[0:64, 0:128], sW2, x1,
                                         start=True, stop=True)
                        x2a = x2a_bufs[t % 2]
                        nc.scalar.activation(x2a[0:64, :], mo2[0:64, 0:128],
                                             Act.Silu, bias=sb2)

                        # --- mm3: per-edge TP weights, 3 PSUM regions ---
                        psA = wpsA.tile([ET, 1536], fp32, tag="A")
                        for c0 in (0, 512, 1024):
                            nc.tensor.matmul(psA[:, c0:c0 + 512], x2a,
                                             sW3[:, c0:c0 + 512],
                                             start=True, stop=True)
                        ps01 = wps01.tile([ET, 512], fp32, tag="o1")
                        nc.tensor.matmul(ps01, x2a, sW3[:, 1536:2048],
                                         start=True, stop=True)
                        psB = wpsB.tile([ET, 256], fp32, tag="B")
                        nc.tensor.matmul(psB, x2a, sW3[:, 2048:2304],
                                         start=True, stop=True)

                        # --- per-edge features ---
                        # fp32 copies of the per-edge scalars (ts needs f32)
                        aux32 = esb.tile([ET, 6], fp32, tag="aux32")
                        nc.gpsimd.tensor_copy(aux32, sv[:, 80:86])
                        # fAV = [se*sh0 (32) | dv (16) | vec*sh0 (48)]
                        # scale-by-partition-scalar runs on ACT (idle)
                        fAV = esb.tile([ET, 96], bf16, tag="fAV")
                        nc.scalar.activation(
                            fAV[:, 0:32], sv[:, 0:32], Act.Copy,
                            scale=aux32[:, 0:1])
                        nc.scalar.activation(
                            fAV[:, 48:96], sv[:, 32:80], Act.Copy,
                            scale=aux32[:, 0:1])
                        t3 = esb.tile([ET, 48], bf16, tag="t3")
                        nc.gpsimd.tensor_tensor(
                            out=t3, in0=sv[:, 32:80],
                            in1=sv[:, 81:84].unsqueeze(2).broadcast_to(
                                (ET, 3, 16)),
                            op=Alu.mult)
                        nc.vector.tensor_reduce(
                            out=fAV[:, 32:48],
                            in_=t3.rearrange("p (c i) -> p i c", c=3),
                            axis=X, op=Alu.add)

                        # --- TP products: DVE straight from PSUM ---
                        prod00 = esb.tile([ET, 1024], bf16, tag="prod00")
                        nc.vector.tensor_tensor(
                            out=prod00.rearrange("p (k o r) -> p k o r",
                                                 k=4, o=32),
                            in0=psA[:, 0:1024].rearrange(
                                "p (k o r) -> p k o r", k=4, o=32),
                            in1=fAV[:, 0:32]
                                .rearrange("p (k r) -> p k r", k=4)
                                .unsqueeze(2).broadcast_to((ET, 4, 32, 8)),
                            op=Alu.mult)
                        prod10 = esb.tile([ET, 512], bf16, tag="prod10")
                        nc.vector.tensor_tensor(
                            out=prod10.rearrange("p (k o r) -> p k o r",
                                                 k=2, o=32),
                            in0=psA[:, 1024:1536].rearrange(
                                "p (k o r) -> p k o r", k=2, o=32),
                            in1=fAV[:, 32:48]
                                .rearrange("p (k r) -> p k r", k=2)
                                .unsqueeze(2).broadcast_to((ET, 2, 32, 8)),
                            op=Alu.mult)
                        fVv = fAV[:, 48:96].rearrange("p (c i) -> p c i",
                                                      c=3)
                        prodBk = []
                        for k in range(2):
                            pk = esb.tile([ET, 384], bf16, tag=f"prodB{k}")
                            nc.vector.tensor_tensor(
                                out=pk.rearrange("p (c o r) -> p c o r",
                                                 c=3, o=16),
                                in0=psB[:, k * 128:(k + 1) * 128]
                                    .rearrange("p (o r) -> p o r", o=16)
                                    .unsqueeze(1)
                                    .broadcast_to((ET, 3, 16, 8)),
                                in1=fVv[:, :, k * 8:(k + 1) * 8]
                                    .unsqueeze(2)
                                    .broadcast_to((ET, 3, 16, 8)),
                                op=Alu.mult)
                            prodBk.append(pk)
                        w01 = esb.tile([ET, 512], bf16, tag="w01")
                        nc.scalar.activation(w01, ps01, Act.Copy)
                        prod01 = esb.tile([ET, 512], bf16, tag="prod01")
                        nc.gpsimd.tensor_tensor(
                            out=prod01.rearrange("p (o i) -> p o i", o=16),
                            in0=w01.rearrange("p (o i) -> p o i", o=16),
                            in1=sv[:, 0:32].unsqueeze(1).broadcast_to(
                                (ET, 16, 32)),
                            op=Alu.mult)

                        # --- fold k-halves: flat contiguous adds ---
                        msgw = esb.tile([ET, 945], bf16, tag="msgw")
                        t00 = esb.tile([ET, 512], bf16, tag="t00")
                        nc.vector.tensor_tensor(
                            out=t00, in0=prod00[:, 0:512],
                            in1=prod00[:, 512:1024], op=Alu.add)
                        nc.vector.tensor_tensor(
                            out=msgw[:, 0:256], in0=t00[:, 0:256],
                            in1=t00[:, 256:512], op=Alu.add)
                        nc.gpsimd.tensor_tensor(
                            out=msgw[:, 256:512], in0=prod10[:, 0:256],
                            in1=prod10[:, 256:512], op=Alu.add)
                        nc.gpsimd.tensor_tensor(
                            out=msgw[:, 512:896], in0=prodBk[0],
                            in1=prodBk[1], op=Alu.add)

                        # --- 01-block: full reduce + sh1 outer product ---
                        p01 = esb.tile([ET, 16], bf16, tag="p01")
                        nc.vector.tensor_reduce(
                            out=p01,
                            in_=prod01.rearrange("p (o i) -> p o i", o=16),
                            axis=X, op=Alu.add)
                        nc.gpsimd.tensor_tensor(
                            out=msgw[:, 896:944].rearrange(
                                "p (c o) -> p c o", c=3),
                            in0=p01.unsqueeze(1).broadcast_to((ET, 3, 16)),
                            in1=sv[:, 81:84].unsqueeze(2).broadcast_to(
                                (ET, 3, 16)),
                            op=Alu.mult)
                        nc.gpsimd.tensor_copy(msgw[:, 944:945], sv[:, 84:85])

                        # --- one-hot scatter matmul (moving dim <= 512) ---
                        # one-hot on ACT: relu(1 - |iota - dstw|); svx col 85
                        # holds -dstw so it can ride the activation bias
                        absd = esb.tile([ET, WIN], bf16, tag="absd")
                        nc.scalar.activation(absd, iotaB, Act.Abs,
                                             bias=aux32[:, 5:6])
                        S = esb.tile([ET, WIN], bf16, tag="S")
                        nc.scalar.activation(S, absd, Act.Relu,
                                             scale=-1.0, bias=1.0)
                        for c0, c1 in ((0, 512), (512, 945)):
                            nc.tensor.matmul(
                                ps_sum[:, c0:c1], S, msgw[:, c0:c1],
                                start=(j == 0), stop=(j == jlast),
                                skip_group_check=True)

                    # ---------------- node phase for window w ----------------
                    # window-level reduce of the scattered 8-wide partials
                    tw = nsb.tile([WIN, 112], fp32, tag="tw")
                    nc.vector.tensor_reduce(
                        out=tw,
                        in_=ps_sum[:, 0:896].rearrange(
                            "p (g r) -> p g r", g=112),
                        axis=X, op=Alu.add)
                    summed = nsb.tile([WIN, 80], fp32, tag="summed")
                    nc.vector.tensor_tensor(
                        out=summed[:, 0:32], in0=tw[:, 0:32],
                        in1=tw[:, 32:64], op=Alu.add)
                    nc.vector.tensor_tensor(
                        out=summed[:, 32:80], in0=tw[:, 64:112],
                        in1=ps_sum[:, 896:944], op=Alu.add)
                    degc = nsb.tile([WIN, 1], fp32, tag="degc")
                    nc.vector.tensor_scalar(
                        degc, ps_sum[:, 944:945], 1.0, None, op0=Alu.max)
                    rdeg = nsb.tile([WIN, 1], fp32, tag="rdeg")
                    nc.vector.reciprocal(rdeg, degc)
                    agg = nsb.tile([WIN, 80], fp32, tag="agg")
                    nc.vector.tensor_scalar(
                        agg, summed, rdeg, None, op0=Alu.mult)

                    # upd -> slab_st[:, w, 0:80]; sq -> [:, w, 80:160]
                    # (self-interaction comes from the SBUF slab, so these
                    # run on Pool; sigmoids are batched in the end phase)
                    nv_v = slab_nv[:, w, 64:160].rearrange(
                        "p (c k) -> p c k", c=3)
                    nc.gpsimd.tensor_tensor(
                        out=slab_st[:, w, 0:32], in0=agg[:, 0:32],
                        in1=slab_nv[:, w, 0:32], op=Alu.add)
                    nc.gpsimd.tensor_tensor(
                        out=slab_st[:, w, 32:80].rearrange(
                            "p (c i) -> p c i", c=3),
                        in0=agg[:, 32:80].rearrange("p (c i) -> p c i", c=3),
                        in1=nv_v[:, :, 0:16], op=Alu.add)
                    nc.gpsimd.tensor_tensor(
                        out=slab_st[:, w, 80:160],
                        in0=slab_st[:, w, 0:80],
                        in1=slab_st[:, w, 0:80], op=Alu.mult)

            # ---------------- end phase: stats, allreduce, BN, update -------
            with (
                tc.tile_pool(name="eps", bufs=1, space="PSUM") as eps_p,
                tc.tile_pool(name="fsb", bufs=1) as fsb,
                tc.tile_pool(name="dram", bufs=2, space="DRAM") as dram,
            ):
                nc.sync.dma_start(out=h_nm, in_=d_hnm[:, :, :])
                # all gate sigmoids in two batched instrs (one table load)
                nc.scalar.activation(
                    slab_g[:, :, 0:32], slab_nv[:, :, 32:64], Act.Sigmoid)
                nc.scalar.activation(
                    slab_g[:, :, 32:80].rearrange(
                        "p w (c i) -> p w c i", c=3),
                    slab_nv[:, :, 64:160].rearrange(
                        "p w (c k) -> p w c k", c=3)[:, :, :, 16:32],
                    Act.Sigmoid)
                ps_st = eps_p.tile([1, 160], fp32, tag="st")
                for w in range(N_WIN):
                    nc.tensor.matmul(
                        ps_st, ones_col, slab_st[:, w, :],
                        start=(w == 0), stop=(w == N_WIN - 1),
                        skip_group_check=True)
                st_sb = fsb.tile([1, 160], fp32, tag="stsb")
                nc.scalar.activation(st_sb, ps_st, Act.Copy)

                ib = dram.tile([1, 160], fp32, tag="ib")
                ob = dram.tile([1, 160], fp32, tag="ob")
                nc.gpsimd.dma_start(ib[:], st_sb[:])
                nc.gpsimd.collective_compute(
                    "AllReduce", mybir.AluOpType.add,
                    replica_groups=[list(range(N_CORES))],
                    ins=[ib.opt()], outs=[ob.opt()])
                st_r = fsb.tile([1, 160], fp32, tag="str")
                nc.gpsimd.dma_start(st_r[:], ob[:])

                inv_n = 1.0 / float(N_NODES)
                meanb = fsb.tile([1, 80], fp32, tag="meanb")
                nc.vector.tensor_scalar(
                    meanb, st_r[:, 0:80], inv_n, None, op0=Alu.mult)
                nc.vector.memset(meanb[:, 32:80], 0.0)
                ex2 = fsb.tile([1, 80], fp32, tag="ex2")
                nc.vector.tensor_scalar(
                    ex2, st_r[:, 80:160], inv_n, None, op0=Alu.mult)
                m2 = fsb.tile([1, 80], fp32, tag="m2")
                nc.vector.tensor_tensor(out=m2, in0=meanb, in1=meanb,
                                        op=Alu.mult)
                exm = fsb.tile([1, 80], fp32, tag="exm")
                nc.vector.tensor_tensor(out=exm, in0=ex2, in1=m2,
                                        op=Alu.subtract)
                vn = fsb.tile([1, 16], fp32, tag="vn")
                nc.vector.tensor_reduce(
                    out=vn,
                    in_=exm[:, 32:80].rearrange("p (c i) -> p i c", c=3),
                    axis=X, op=Alu.add)
                varb = fsb.tile([1, 80], fp32, tag="varb")
                nc.vector.tensor_scalar(
                    varb[:, 0:32], exm[:, 0:32], 1.0, float(EPS),
                    op0=Alu.mult, op1=Alu.add)
                nc.vector.tensor_scalar(
                    varb[:, 32:80].rearrange("p (c i) -> p c i", c=3),
                    vn.unsqueeze(1).broadcast_to((1, 3, 16)),
                    1.0 / 3.0, float(EPS), op0=Alu.mult, op1=Alu.add)
                rec = fsb.tile([1, 80], fp32, tag="rec")
                nc.vector.reciprocal(rec, varb)
                rstd = fsb.tile([1, 80], fp32, tag="rstd")
                nc.scalar.activation(rstd, rec, Act.Sqrt)
                scsh = fsb.tile([1, 160], fp32, tag="scsh")
                nc.vector.tensor_tensor(
                    out=scsh[:, 0:80], in0=rstd, in1=bnw, op=Alu.mult)
                msc = fsb.tile([1, 80], fp32, tag="msc")
                nc.vector.tensor_tensor(
                    out=msc, in0=meanb, in1=scsh[:, 0:80], op=Alu.mult)
                nc.vector.tensor_tensor(
                    out=scsh[:, 80:160], in0=bnb, in1=msc, op=Alu.subtract)

                ps_b = eps_p.tile([128, 160], fp32, tag="bc")
                nc.tensor.matmul(ps_b, ones_row, scsh, start=True, stop=True)
                scshB = fsb.tile([128, 160], fp32, tag="scshB")
                nc.scalar.activation(scshB, ps_b, Act.Copy)

                # batched gated residual update over the whole node slab:
                # out = (upd*sc + sh) * g + h, broadcasting sc/sh per window
                scB = scshB[:, 0:80].unsqueeze(1).broadcast_to(
                    (WIN, N_WIN, 80))
                shB = scshB[:, 80:160].unsqueeze(1).broadcast_to(
                    (WIN, N_WIN, 80))
                t1 = fsb.tile([WIN, N_WIN, 80], fp32, tag="t1")
                nc.vector.tensor_tensor(
                    out=t1, in0=slab_st[:, :, 0:80], in1=scB, op=Alu.mult)
                t2 = fsb.tile([WIN, N_WIN, 80], fp32, tag="t2")
                nc.vector.tensor_tensor(
                    out=t2, in0=t1, in1=shB, op=Alu.add)
                nc.vector.tensor_tensor(
                    out=t1, in0=t2, in1=slab_g, op=Alu.mult)
                nc.vector.tensor_tensor(
                    out=slab_out, in0=t1, in1=h_nm, op=Alu.add)
                nc.sync.dma_start(out=d_out[:, :, :], in_=slab_out)
    nc.compile()
    return nc


def kernel(**inputs):
    import os
    from concourse.bass_utils import run_bass_kernel_spmd
    import ml_dtypes

    bf16 = ml_dtypes.bfloat16
    trace = os.environ.get("KERNEL_TRACE", "0") == "1"
    inputs = {k: np.asarray(v) for k, v in inputs.items()}
    edge_index = inputs["edge_index"].astype(np.int64)
    inputs["edge_index"] = edge_index
    for k in list(inputs):
        if inputs[k].dtype == np.float64:
            inputs[k] = inputs[k].astype(np.float32)

    order, tiles_per_win, win_of, pos_of = _shard(edge_index)

    # shared weights (host-side dtype conversion / packing)
    W3b = np.vstack([
        inputs["W3"].astype(np.float32),
        inputs["b3"].astype(np.float32).reshape(1, WNUM),
    ])
    W3p = _w3_permute(W3b).astype(bf16)
    Wn = np.zeros((33, 64), np.float32)
    Wn[0, 0:32] = inputs["bs_s"]
    Wn[1:, 0:32] = inputs["ws_s"]
    Wn[0, 32:64] = inputs["bg_s"]
    Wn[1:, 32:64] = inputs["wg_s"]
    Wv = np.zeros((16, 32), np.float32)
    Wv[:, 0:16] = inputs["ws_v"]
    Wv[:, 16:32] = inputs["wg_v"]
    # one block-diagonal weight so each window's whole self-interaction +
    # gate pre-activation is a single [81,128]@[81,160] matmul
    W_all = np.zeros((81, 160), np.float32)
    W_all[0:33, 0:64] = Wn
    for cc in range(3):
        W_all[33 + cc * 16:49 + cc * 16, 64 + cc * 32:96 + cc * 32] = Wv
    bnw_row = np.zeros((1, 80), np.float32)
    bnw_row[0, 0:32] = inputs["bn_ws"]
    for cc in range(3):
        bnw_row[0, 32 + cc * 16:48 + cc * 16] = inputs["bn_wv"]
    bnb_row = np.zeros((1, 80), np.float32)
    bnb_row[0, 0:32] = inputs["bn_bs"]

    core_maps = []
    T = None
    for c in range(N_CORES):
        m, T = _build_core_inputs(
            inputs, order, tiles_per_win, win_of, pos_of, c, bf16)
        m["W1"] = inputs["W1"].astype(bf16)
        m["b1"] = inputs["b1"].astype(np.float32).reshape(64, 1)
        m["W2"] = inputs["W2"].astype(bf16)
        m["b2"] = inputs["b2"].astype(np.float32).reshape(64, 1)
        m["W3"] = W3p
        m["W_all"] = W_all.astype(bf16)
        m["bnw"] = bnw_row
        m["bnb"] = bnb_row
        core_maps.append(m)

    nc1 = _fused_bass(T, tiles_per_win)
    r1 = run_bass_kernel_spmd(
        nc1, core_maps, core_ids=list(range(N_CORES)), trace=trace)
    global _last_exec_ns
    _last_exec_ns = r1.exec_time_ns
    _last_rr.clear()
    _last_rr.append(r1)

    out = np.zeros((N_NODES, 80), np.float32)
    for c in range(N_CORES):
        slab = r1.results[c]["out"]          # [WIN, N_WIN, 80]
        blk = slab[pos_of[c], win_of[c]]     # [2500, 80] internal layout
        out[c * NPC:(c + 1) * NPC] = blk[:, _perm_int2ref]
    return out


if __name__ == "__main__":
    import reference

    inp = reference.setup_inputs()
    inp = {k: np.asarray(v) for k, v in inp.items()}
    expected = np.asarray(reference.reference(**inp))
    actual = kernel(**inp)
    err = np.abs(actual - expected)
    rel = np.linalg.norm(actual - expected) / np.linalg.norm(expected)
    print("max abs err:", err.max(), "rel:", rel)


# revision 51
# speedup vs baseline: 8.7600x; 1.0262x over previous
"""Trainium2 Bass kernel for nn_EquivariantBlock (gnn_message_passing).

Single fused kernel, nodes partitioned across 8 cores (2500 each).
Host does sharding/gather/layout only; all FLOPs run on device.

Per core: edges grouped by destination window (128-node windows, node->window
assignment load-balanced via LPT so every window needs ~the same tile count).
Edge phase per 128-edge tile: bf16 edge MLP on PE -> per-edge TP weights in
PSUM (never touch HBM; W3 columns pre-permuted + CG/alpha scales folded on
host) -> ACT evacuates weights to SBUF bf16 -> DVE forms per-edge products
(plain tensor_tensor, 2x bf16 mode) -> Pool engine reduces -> one-hot scatter
matmul accumulates per-window sums in PSUM.  Node phase interleaved per
window: degree-mean, self-interaction + gate matmuls (bf16), batch-stat slab.
End phase: stats matmul -> 640B AllReduce across the 8 cores -> BN scale/shift
vectors on-device -> broadcast via matmul -> gated residual update -> one DMA.
"""

import numpy as np

MUL0, MUL1 = 32, 16
EDGE_DIM, HID = 32, 64
WNUM = 2304
N_NODES, N_EDGES = 20000, 100000
EPS = 1e-5
ALPHA = 1.0 / np.sqrt(48.0)
INV_SQRT3 = 1.0 / np.sqrt(3.0)

N_CORES = 8
NPC = N_NODES // N_CORES          # nodes per core = 2500
N_WIN = 20                        # windows per core
NPW = NPC // N_WIN                # nodes per window = 125 (< 128)
WIN = 128                         # window slot size (partition dim)
NPAD = N_WIN * WIN                # 2560
ET = 128                          # edges per tile

_last_exec_ns = None
_last_rr = []

# column permutation: reference h layout [s(32), v (i-major: i*3+c)]
# internal layout  [s(32), v (c-major: c*16+i)]
_perm_ref2int = np.concatenate(
    [np.arange(32)] + [32 + np.arange(16) * 3 + c for c in range(3)]
).astype(np.int64)
_perm_int2ref = np.argsort(_perm_ref2int)


def _w3_permute(W3b):
    """Permute + scale the (W3;b3) columns for the on-device TP layout.

    Input rows [65, 2304] in reference order:
      w00 col i*32+o, w10 1024+i*32+o, w01 1536+i*16+o, w11 2048+i*16+o.

    Output layout, scales folded.  Blocks 00/10/11 use a k-interleaved
    order (i = k*8 + r) so the i-contraction runs as flat contiguous
    half-adds on DVE followed by an 8-wide window-level reduce:
      00 region       k*256 + o*8 + r (k<4,o<32,r<8): alpha*w00[k*8+r, o]
      10 region 1024+ k*256 + o*8 + r (k<2,o<32,r<8): a/sqrt3*w10[k*8+r,o]
      01 region 1536+ o*32 + i       (o<16,i<32):     alpha*w01[i,o]
      11 region 2048+ k*128 + o*8 + r (k<2,o<16,r<8): alpha*w11[k*8+r,o]
    """
    idx = np.empty(WNUM, np.int64)
    scl = np.empty(WNUM, np.float32)
    for k in range(4):
        for o in range(32):
            for r in range(8):
                idx[k * 256 + o * 8 + r] = (k * 8 + r) * 32 + o
                scl[k * 256 + o * 8 + r] = ALPHA
    for k in range(2):
        for o in range(32):
            for r in range(8):
                idx[1024 + k * 256 + o * 8 + r] = 1024 + (k * 8 + r) * 32 + o
                scl[1024 + k * 256 + o * 8 + r] = ALPHA * INV_SQRT3
    for o in range(16):
        for i in range(32):
            idx[1536 + o * 32 + i] = 1536 + i * 16 + o
            scl[1536 + o * 32 + i] = ALPHA
    for k in range(2):
        for o in range(16):
            for r in range(8):
                idx[2048 + k * 128 + o * 8 + r] = 2048 + (k * 8 + r) * 16 + o
                scl[2048 + k * 128 + o * 8 + r] = ALPHA
    return (W3b[:, idx] * scl[None, :]).astype(np.float32)


def _balance_nodes(dst):
    """Assign each core's local nodes to N_WIN windows (<=NPW nodes each),
    balancing per-window edge counts (greedy LPT).  Returns win_of, pos_of
    [N_CORES, NPC] and per-(core,window) edge counts."""
    core = dst // NPC
    dloc = dst - core * NPC
    win_of = np.zeros((N_CORES, NPC), np.int64)
    pos_of = np.zeros((N_CORES, NPC), np.int64)
    ecnt = np.zeros((N_CORES, N_WIN), np.int64)
    for c in range(N_CORES):
        deg = np.bincount(dloc[core == c], minlength=NPC)
        order = np.argsort(-deg, kind="stable")
        loads = np.zeros(N_WIN, np.int64)
        counts = np.zeros(N_WIN, np.int64)
        for n in order:
            open_w = np.nonzero(counts < NPW)[0]
            w = open_w[np.argmin(loads[open_w])]
            win_of[c, n] = w
            pos_of[c, n] = counts[w]
            counts[w] += 1
            loads[w] += deg[n]
        ecnt[c] = loads
    return win_of, pos_of, ecnt


def _shard(edge_index):
    src, dst = edge_index[0], edge_index[1]
    win_of, pos_of, ecnt = _balance_nodes(dst)
    tiles_per_win = [
        max(1, int(max((ecnt[c, w] + ET - 1) // ET for c in range(N_CORES))))
        for w in range(N_WIN)
    ]
    core = dst // NPC
    dloc = dst - core * NPC
    order = [[None] * N_WIN for _ in range(N_CORES)]
    for c in range(N_CORES):
        idx = np.nonzero(core == c)[0]
        w_of = win_of[c][dloc[idx]]
        s = np.argsort(w_of, kind="stable")
        idx = idx[s]
        w_of = w_of[s]
        bounds = np.searchsorted(w_of, np.arange(N_WIN + 1))
        for w in range(N_WIN):
            order[c][w] = idx[bounds[w]:bounds[w + 1]]
    return order, tiles_per_win, win_of, pos_of


def _build_core_inputs(inputs, order, tiles_per_win, win_of, pos_of, c, bf16):
    h = inputs["h"]
    edge_sh = inputs["edge_sh"]
    ef = inputs["edge_features"]
    src = inputs["edge_index"][0]
    dst = inputs["edge_index"][1]

    T = int(sum(tiles_per_win))
    E_pad = T * ET
    svx = np.zeros((E_pad, 96), np.float32)
    efp = np.zeros((E_pad, EDGE_DIM), np.float32)

    pos = 0
    for w in range(N_WIN):
        ids = order[c][w]
        n = len(ids)
        sl = slice(pos, pos + n)
        hs = h[src[ids]][:, _perm_ref2int]     # [n, 80] internal layout
        svx[sl, 0:80] = hs
        svx[sl, 80] = edge_sh[ids, 0]
        svx[sl, 81:84] = edge_sh[ids, 1:4]
        svx[sl, 84] = 1.0
        svx[sl, 85] = -pos_of[c][dst[ids] - c * NPC].astype(np.float32)
        efp[sl] = ef[ids]
        pos += tiles_per_win[w] * ET
    efT = np.ascontiguousarray(
        efp.reshape(T, ET, EDGE_DIM).transpose(0, 2, 1)
    )

    # node-side layouts follow the per-core window permutation
    hsl = h[c * NPC:(c + 1) * NPC][:, _perm_ref2int]   # [2500, 80] internal
    col = win_of[c] * WIN + pos_of[c]                  # node -> slab column
    hT_all = np.zeros((81, NPAD), np.float32)
    hT_all[0, col] = 1.0
    hT_all[1:33, col] = hsl[:, :32].T
    for cc in range(3):
        hT_all[33 + cc * 16:49 + cc * 16, col] = \
            hsl[:, 32 + cc * 16:32 + (cc + 1) * 16].T
    h_nm = np.zeros((WIN, N_WIN, 80), np.float32)      # node-major slab
    h_nm[pos_of[c], win_of[c]] = hsl

    return dict(
        svx=svx.reshape(T, ET, 96).astype(bf16),
        efT=efT.astype(bf16),
        hT_all=hT_all.astype(bf16),
        h_nm=h_nm,
    ), T


def _fused_bass(T, tiles_per_win):
    import concourse.bacc as bacc
    import concourse.mybir as mybir
    import concourse.tile as tile

    fp32 = mybir.dt.float32
    bf16 = mybir.dt.bfloat16
    Alu = mybir.AluOpType
    Act = mybir.ActivationFunctionType
    X = mybir.AxisListType.X

    nc = bacc.Bacc("TRN2", target_bir_lowering=False, debug=False,
                   num_devices=N_CORES)
    d_svx = nc.dram_tensor("svx", [T, ET, 96], bf16, kind="ExternalInput")
    d_efT = nc.dram_tensor("efT", [T, 32, ET], bf16, kind="ExternalInput")
    d_hall = nc.dram_tensor("hT_all", [81, NPAD], bf16,
                            kind="ExternalInput")
    d_hnm = nc.dram_tensor("h_nm", [WIN, N_WIN, 80], fp32,
                           kind="ExternalInput")
    d_W1 = nc.dram_tensor("W1", [32, 64], bf16, kind="ExternalInput")
    d_b1 = nc.dram_tensor("b1", [64, 1], fp32, kind="ExternalInput")
    d_W2 = nc.dram_tensor("W2", [64, 64], bf16, kind="ExternalInput")
    d_b2 = nc.dram_tensor("b2", [64, 1], fp32, kind="ExternalInput")
    d_W3 = nc.dram_tensor("W3", [65, WNUM], bf16, kind="ExternalInput")
    d_wall = nc.dram_tensor("W_all", [81, 160], bf16, kind="ExternalInput")
    d_bnw = nc.dram_tensor("bnw", [1, 80], fp32, kind="ExternalInput")
    d_bnb = nc.dram_tensor("bnb", [1, 80], fp32, kind="ExternalInput")
    d_out = nc.dram_tensor("out", [WIN, N_WIN, 80], fp32,
                           kind="ExternalOutput")

    with tile.TileContext(nc) as tc, \
            nc.allow_low_precision(reason="bf16 TP well within 2e-2 tol"):
        with tc.tile_pool(name="singles", bufs=1) as singles:
            sW1 = singles.tile([32, 64], bf16)
            nc.sync.dma_start(out=sW1, in_=d_W1[:, :])
            sb1 = singles.tile([64, 1], fp32)
            nc.sync.dma_start(out=sb1, in_=d_b1[:, :])
            sW2 = singles.tile([64, 64], bf16)
            nc.sync.dma_start(out=sW2, in_=d_W2[:, :])
            sb2 = singles.tile([64, 1], fp32)
            nc.sync.dma_start(out=sb2, in_=d_b2[:, :])
            sW3 = singles.tile([65, WNUM], bf16)
            nc.sync.dma_start(out=sW3, in_=d_W3[:, :])
            sWall = singles.tile([81, 160], bf16)
            nc.sync.dma_start(out=sWall, in_=d_wall[:, :])
            sHall = singles.tile([81, NPAD], bf16)
            nc.sync.dma_start(out=sHall, in_=d_hall[:, :])
            # h_nm is only read in the end phase; DMA issued there so the
            # 820KB transfer doesn't stall the first edge tiles' loads
            h_nm = singles.tile([WIN, N_WIN, 80], fp32)
            bnw = singles.tile([1, 80], fp32)
            nc.sync.dma_start(out=bnw, in_=d_bnw[:, :])
            bnb = singles.tile([1, 80], fp32)
            nc.sync.dma_start(out=bnb, in_=d_bnb[:, :])

            iotaI = singles.tile([ET, WIN], mybir.dt.int32)
            nc.gpsimd.iota(iotaI, [[1, WIN]], channel_multiplier=0)
            iotaB = singles.tile([ET, WIN], bf16)
            nc.vector.tensor_copy(iotaB, iotaI)
            ones_col = singles.tile([WIN, 1], bf16)
            nc.vector.memset(ones_col, 1.0)
            ones_row = singles.tile([1, WIN], fp32)
            nc.vector.memset(ones_row, 1.0)

            slab_st = singles.tile([WIN, N_WIN, 160], bf16)
            slab_g = singles.tile([WIN, N_WIN, 80], bf16)
            slab_out = singles.tile([WIN, N_WIN, 80], fp32)

            # manual double-buffer for x2a: row 64 is a constant ones row
            # (the b3 contraction row), written once — pools would force a
            # per-tile rewrite.
            x2a_bufs = []
            for i in range(2):
                xt = singles.tile([65, ET], bf16, tag=f"x2a{i}")
                nc.vector.memset(xt[64:65, :], 1.0)
                x2a_bufs.append(xt)

            # self-interaction + gate pre-acts for ALL windows, in a
            # dedicated pre-scope with 4 PSUM banks (free before the edge
            # pools open) so the 20 matmuls pipeline instead of ping-ponging
            # through one bank, overlapping the edge-phase DMA fill.
            # Layout per window: [si_s(32)|gate_s(32)|3 x (si_v(16)|gate_v(16))]
            slab_nv = singles.tile([WIN, N_WIN, 160], fp32)
            with tc.tile_pool(name="hps", bufs=4, space="PSUM") as hps:
                for w in range(N_WIN):
                    nmo = hps.tile([128, 160], fp32, tag="hmm")
                    nc.tensor.matmul(nmo,
                                     sHall[:, w * WIN:(w + 1) * WIN], sWall,
                                     start=True, stop=True)
                    nc.scalar.activation(slab_nv[:, w, :], nmo, Act.Copy)

            # ---------------- edge + node phase (per window) ----------------
            # scatter payload layout (945 bf16 cols per edge):
            #   0:256    00-block partial sums, (o=32, r=8)
            #   256:512  10-block partial sums, (o=32, r=8)
            #   512:896  11-block partial sums, (c=3, o=16, r=8)
            #   896:944  p01*sh1, (c=3, o=16)
            #   944      valid (degree)
            with (
                tc.tile_pool(name="edma", bufs=6) as edma,
                tc.tile_pool(name="esb", bufs=3) as esb,
                tc.tile_pool(name="nsb", bufs=2) as nsb,
                tc.tile_pool(name="wpsA", bufs=1, space="PSUM") as wpsA,
                tc.tile_pool(name="wps01", bufs=1, space="PSUM") as wps01,
                tc.tile_pool(name="wpsB", bufs=1, space="PSUM") as wpsB,
                tc.tile_pool(name="mmout", bufs=1, space="PSUM") as mmout,
                tc.tile_pool(name="sps", bufs=1, space="PSUM") as sps,
            ):
                t_idx = 0
                for w in range(N_WIN):
                    ps_sum = sps.tile([WIN, 945], fp32, tag="scat")
                    jlast = tiles_per_win[w] - 1
                    for j in range(tiles_per_win[w]):
                        t = t_idx
                        t_idx += 1
                        sv = edma.tile([ET, 96], bf16, tag="svx")
                        nc.sync.dma_start(out=sv, in_=d_svx[t, :, :])
                        ef_t = edma.tile([32, ET], bf16, tag="ef")
                        nc.sync.dma_start(out=ef_t, in_=d_efT[t, :, :])

                        # --- edge MLP (feature-major, bf16) ---
                        mo1 = mmout.tile([128, 160], fp32, tag="mm")
                        nc.tensor.matmul(mo1[0:64, 0:128], sW1, ef_t,
                                         start=True, stop=True)
                        x1 = esb.tile([64, ET], bf16, tag="x1")
                        nc.scalar.activation(x1, mo1[0:64, 0:128], Act.Silu,
                                             bias=sb1)
                        mo2 = mmout.tile([128, 160], fp32, tag="mm")
                        nc.tensor.matmul(mo2[0:64, 0:128], sW2, x1,
                                         start=True, stop=True)
                        x2a = x2a_bufs[t % 2]
                        nc.scalar.activation(x2a[0:64, :], mo2[0:64, 0:128],
                                             Act.Silu, bias=sb2)

                        # --- mm3: per-edge TP weights, 3 PSUM regions ---
                        psA = wpsA.tile([ET, 1536], fp32, tag="A")
                        for c0 in (0, 512, 1024):
                            nc.tensor.matmul(psA[:, c0:c0 + 512], x2a,
                                             sW3[:, c0:c0 + 512],
                                             start=True, stop=True)
                        ps01 = wps01.tile([ET, 512], fp32, tag="o1")
                        nc.tensor.matmul(ps01, x2a, sW3[:, 1536:2048],
                                         start=True, stop=True)
                        psB = wpsB.tile([ET, 256], fp32, tag="B")
                        nc.tensor.matmul(psB, x2a, sW3[:, 2048:2304],
                                         start=True, stop=True)

                        # --- per-edge features ---
                        # fp32 copies of the per-edge scalars (ts needs f32)
                        aux32 = esb.tile([ET, 6], fp32, tag="aux32")
                        nc.gpsimd.tensor_copy(aux32, sv[:, 80:86])
                        # fAV = [se*sh0 (32) | dv (16) | vec*sh0 (48)]
                        # scale-by-partition-scalar runs on ACT (idle)
                        fAV = esb.tile([ET, 96], bf16, tag="fAV")
                        nc.scalar.activation(
                            fAV[:, 0:32], sv[:, 0:32], Act.Copy,
                            scale=aux32[:, 0:1])
                        nc.scalar.activation(
                            fAV[:, 48:96], sv[:, 32:80], Act.Copy,
                            scale=aux32[:, 0:1])
                        t3 = esb.tile([ET, 48], bf16, tag="t3")
                        nc.gpsimd.tensor_tensor(
                            out=t3, in0=sv[:, 32:80],
                            in1=sv[:, 81:84].unsqueeze(2).broadcast_to(
                                (ET, 3, 16)),
                            op=Alu.mult)
                        nc.vector.tensor_reduce(
                            out=fAV[:, 32:48],
                            in_=t3.rearrange("p (c i) -> p i c", c=3),
                            axis=X, op=Alu.add)

                        # --- TP products: DVE straight from PSUM ---
                        prod00 = esb.tile([ET, 1024], bf16, tag="prod00")
                        nc.vector.tensor_tensor(
                            out=prod00.rearrange("p (k o r) -> p k o r",
                                                 k=4, o=32),
                            in0=psA[:, 0:1024].rearrange(
                                "p (k o r) -> p k o r", k=4, o=32),
                            in1=fAV[:, 0:32]
                                .rearrange("p (k r) -> p k r", k=4)
                                .unsqueeze(2).broadcast_to((ET, 4, 32, 8)),
                            op=Alu.mult)
                        prod10 = esb.tile([ET, 512], bf16, tag="prod10")
                        nc.vector.tensor_tensor(
                            out=prod10.rearrange("p (k o r) -> p k o r",
                                                 k=2, o=32),
                            in0=psA[:, 1024:1536].rearrange(
                                "p (k o r) -> p k o r", k=2, o=32),
                            in1=fAV[:, 32:48]
                                .rearrange("p (k r) -> p k r", k=2)
                                .unsqueeze(2).broadcast_to((ET, 2, 32, 8)),
                            op=Alu.mult)
                        fVv = fAV[:, 48:96].rearrange("p (c i) -> p c i",
                                                      c=3)
                        prodBk = []
                        for k in range(2):
                            pk = esb.tile([ET, 384], bf16, tag=f"prodB{k}")
                            nc.vector.tensor_tensor(
                                out=pk.rearrange("p (c o r) -> p c o r",
                                                 c=3, o=16),
                                in0=psB[:, k * 128:(k + 1) * 128]
                                    .rearrange("p (o r) -> p o r", o=16)
                                    .unsqueeze(1)
                                    .broadcast_to((ET, 3, 16, 8)),
                                in1=fVv[:, :, k * 8:(k + 1) * 8]
                                    .unsqueeze(2)
                                    .broadcast_to((ET, 3, 16, 8)),
                                op=Alu.mult)
                            prodBk.append(pk)
                        w01 = esb.tile([ET, 512], bf16, tag="w01")
                        nc.scalar.activation(w01, ps01, Act.Copy)
                        prod01 = esb.tile([ET, 512], bf16, tag="prod01")
                        nc.gpsimd.tensor_tensor(
                            out=prod01.rearrange("p (o i) -> p o i", o=16),
                            in0=w01.rearrange("p (o i) -> p o i", o=16),
                            in1=sv[:, 0:32].unsqueeze(1).broadcast_to(
                                (ET, 16, 32)),
                            op=Alu.mult)

                        # --- fold k-halves: flat contiguous adds ---
                        msgw = esb.tile([ET, 945], bf16, tag="msgw")
                        t00 = esb.tile([ET, 512], bf16, tag="t00")
                        nc.vector.tensor_tensor(
                            out=t00, in0=prod00[:, 0:512],
                            in1=prod00[:, 512:1024], op=Alu.add)
                        nc.vector.tensor_tensor(
                            out=msgw[:, 0:256], in0=t00[:, 0:256],
                            in1=t00[:, 256:512], op=Alu.add)
                        nc.gpsimd.tensor_tensor(
                            out=msgw[:, 256:512], in0=prod10[:, 0:256],
                            in1=prod10[:, 256:512], op=Alu.add)
                        nc.gpsimd.tensor_tensor(
                            out=msgw[:, 512:896], in0=prodBk[0],
                            in1=prodBk[1], op=Alu.add)

                        # --- 01-block: full reduce + sh1 outer product ---
                        p01 = esb.tile([ET, 16], bf16, tag="p01")
                        nc.vector.tensor_reduce(
                            out=p01,
                            in_=prod01.rearrange("p (o i) -> p o i", o=16),
                            axis=X, op=Alu.add)
                        nc.gpsimd.tensor_tensor(
                            out=msgw[:, 896:944].rearrange(
                                "p (c o) -> p c o", c=3),
                            in0=p01.unsqueeze(1).broadcast_to((ET, 3, 16)),
                            in1=sv[:, 81:84].unsqueeze(2).broadcast_to(
                                (ET, 3, 16)),
                            op=Alu.mult)
                        nc.gpsimd.tensor_copy(msgw[:, 944:945], sv[:, 84:85])

                        # --- one-hot scatter matmul (moving dim <= 512) ---
                        # one-hot on ACT: relu(1 - |iota - dstw|); svx col 85
                        # holds -dstw so it can ride the activation bias
                        absd = esb.tile([ET, WIN], bf16, tag="absd")
                        nc.scalar.activation(absd, iotaB, Act.Abs,
                                             bias=aux32[:, 5:6])
                        S = esb.tile([ET, WIN], bf16, tag="S")
                        nc.scalar.activation(S, absd, Act.Relu,
                                             scale=-1.0, bias=1.0)
                        for c0, c1 in ((0, 512), (512, 945)):
                            nc.tensor.matmul(
                                ps_sum[:, c0:c1], S, msgw[:, c0:c1],
                                start=(j == 0), stop=(j == jlast),
                                skip_group_check=True)

                    # ---------------- node phase for window w ----------------
                    # window-level reduce of the scattered 8-wide partials
                    tw = nsb.tile([WIN, 112], fp32, tag="tw")
                    nc.vector.tensor_reduce(
                        out=tw,
                        in_=ps_sum[:, 0:896].rearrange(
                            "p (g r) -> p g r", g=112),
                        axis=X, op=Alu.add)
                    summed = nsb.tile([WIN, 80], fp32, tag="summed")
                    nc.vector.tensor_tensor(
                        out=summed[:, 0:32], in0=tw[:, 0:32],
                        in1=tw[:, 32:64], op=Alu.add)
                    nc.vector.tensor_tensor(
                        out=summed[:, 32:80], in0=tw[:, 64:112],
                        in1=ps_sum[:, 896:944], op=Alu.add)
                    degc = nsb.tile([WIN, 1], fp32, tag="degc")
                    nc.vector.tensor_scalar(
                        degc, ps_sum[:, 944:945], 1.0, None, op0=Alu.max)
                    rdeg = nsb.tile([WIN, 1], fp32, tag="rdeg")
                    nc.vector.reciprocal(rdeg, degc)
                    agg = nsb.tile([WIN, 80], fp32, tag="agg")
                    nc.vector.tensor_scalar(
                        agg, summed, rdeg, None, op0=Alu.mult)

                    # upd -> slab_st[:, w, 0:80]; sq -> [:, w, 80:160]
                    # (self-interaction comes from the SBUF slab, so these
                    # run on Pool; sigmoids are batched in the end phase)
                    nv_v = slab_nv[:, w, 64:160].rearrange(
                        "p (c k) -> p c k", c=3)
                    nc.gpsimd.tensor_tensor(
                        out=slab_st[:, w, 0:32], in0=agg[:, 0:32],
                        in1=slab_nv[:, w, 0:32], op=Alu.add)
                    nc.gpsimd.tensor_tensor(
                        out=slab_st[:, w, 32:80].rearrange(
                            "p (c i) -> p c i", c=3),
                        in0=agg[:, 32:80].rearrange("p (c i) -> p c i", c=3),
                        in1=nv_v[:, :, 0:16], op=Alu.add)
                    nc.gpsimd.tensor_tensor(
                        out=slab_st[:, w, 80:160],
                        in0=slab_st[:, w, 0:80],
                        in1=slab_st[:, w, 0:80], op=Alu.mult)

            # ---------------- end phase: stats, allreduce, BN, update -------
            with (
                tc.tile_pool(name="eps", bufs=1, space="PSUM") as eps_p,
                tc.tile_pool(name="fsb", bufs=1) as fsb,
                tc.tile_pool(name="dram", bufs=2, space="DRAM") as dram,
            ):
                nc.sync.dma_start(out=h_nm, in_=d_hnm[:, :, :])
                # all gate sigmoids in two batched instrs (one table load)
                nc.scalar.activation(
                    slab_g[:, :, 0:32], slab_nv[:, :, 32:64], Act.Sigmoid)
                nc.scalar.activation(
                    slab_g[:, :, 32:80].rearrange(
                        "p w (c i) -> p w c i", c=3),
                    slab_nv[:, :, 64:160].rearrange(
                        "p w (c k) -> p w c k", c=3)[:, :, :, 16:32],
                    Act.Sigmoid)
                ps_st = eps_p.tile([1, 160], fp32, tag="st")
                for w in range(N_WIN):
                    nc.tensor.matmul(
                        ps_st, ones_col, slab_st[:, w, :],
                        start=(w == 0), stop=(w == N_WIN - 1),
                        skip_group_check=True)
                st_sb = fsb.tile([1, 160], fp32, tag="stsb")
                nc.scalar.activation(st_sb, ps_st, Act.Copy)

                ib = dram.tile([1, 160], fp32, tag="ib")
                ob = dram.tile([1, 160], fp32, tag="ob")
                nc.gpsimd.dma_start(ib[:], st_sb[:])
                nc.gpsimd.collective_compute(
                    "AllReduce", mybir.AluOpType.add,
                    replica_groups=[list(range(N_CORES))],
                    ins=[ib.opt()], outs=[ob.opt()])
                st_r = fsb.tile([1, 160], fp32, tag="str")
                nc.gpsimd.dma_start(st_r[:], ob[:])

                inv_n = 1.0 / float(N_NODES)
                meanb = fsb.tile([1, 80], fp32, tag="meanb")
                nc.vector.tensor_scalar(
                    meanb, st_r[:, 0:80], inv_n, None, op0=Alu.mult)
                nc.vector.memset(meanb[:, 32:80], 0.0)
                ex2 = fsb.tile([1, 80], fp32, tag="ex2")
                nc.vector.tensor_scalar(
                    ex2, st_r[:, 80:160], inv_n, None, op0=Alu.mult)
                m2 = fsb.tile([1, 80], fp32, tag="m2")
                nc.vector.tensor_tensor(out=m2, in0=meanb, in1=meanb,
                                        op=Alu.mult)
                exm = fsb.tile([1, 80], fp32, tag="exm")
                nc.vector.tensor_tensor(out=exm, in0=ex2, in1=m2,
                                        op=Alu.subtract)
                vn = fsb.tile([1, 16], fp32, tag="vn")
                nc.vector.tensor_reduce(
                    out=vn,
                    in_=exm[:, 32:80].rearrange("p (c i) -> p i c", c=3),
                    axis=X, op=Alu.add)
                varb = fsb.tile([1, 80], fp32, tag="varb")
                nc.vector.tensor_scalar(
                    varb[:, 0:32], exm[:, 0:32], 1.0, float(EPS),
                    op0=Alu.mult, op1=Alu.add)
                nc.vector.tensor_scalar(
                    varb[:, 32:80].rearrange("p (c i) -> p c i", c=3),
                    vn.unsqueeze(1).broadcast_to((1, 3, 16)),
                    1.0 / 3.0, float(EPS), op0=Alu.mult, op1=Alu.add)
                rec = fsb.tile([1, 80], fp32, tag="rec")
                nc.vector.reciprocal(rec, varb)
                rstd = fsb.tile([1, 80], fp32, tag="rstd")
                nc.scalar.activation(rstd, rec, Act.Sqrt)
                scsh = fsb.tile([1, 160], fp32, tag="scsh")
                nc.vector.tensor_tensor(
                    out=scsh[:, 0:80], in0=rstd, in1=bnw, op=Alu.mult)
                msc = fsb.tile([1, 80], fp32, tag="msc")
                nc.vector.tensor_tensor(
                    out=msc, in0=meanb, in1=scsh[:, 0:80], op=Alu.mult)
                nc.vector.tensor_tensor(
                    out=scsh[:, 80:160], in0=bnb, in1=msc, op=Alu.subtract)

                ps_b = eps_p.tile([128, 160], fp32, tag="bc")
                nc.tensor.matmul(ps_b, ones_row, scsh, start=True, stop=True)
                scshB = fsb.tile([128, 160], fp32, tag="scshB")
                nc.scalar.activation(scshB, ps_b, Act.Copy)

                # batched gated residual update over the whole node slab:
                # out = (upd*sc + sh) * g + h, broadcasting sc/sh per window
                scB = scshB[:, 0:80].unsqueeze(1).broadcast_to(
                    (WIN, N_WIN, 80))
                shB = scshB[:, 80:160].unsqueeze(1).broadcast_to(
                    (WIN, N_WIN, 80))
                t1 = fsb.tile([WIN, N_WIN, 80], fp32, tag="t1")
                nc.vector.tensor_tensor(
                    out=t1, in0=slab_st[:, :, 0:80], in1=scB, op=Alu.mult)
                t2 = fsb.tile([WIN, N_WIN, 80], fp32, tag="t2")
                nc.vector.tensor_tensor(
                    out=t2, in0=t1, in1=shB, op=Alu.add)
                nc.vector.tensor_tensor(
                    out=t1, in0=t2, in1=slab_g, op=Alu.mult)
                nc.vector.tensor_tensor(
                    out=slab_out, in0=t1, in1=h_nm, op=Alu.add)
                nc.sync.dma_start(out=d_out[:, :, :], in_=slab_out)
    nc.compile()
    return nc


def kernel(**inputs):
    import os
    from concourse.bass_utils import run_bass_kernel_spmd
    import ml_dtypes

    bf16 = ml_dtypes.bfloat16
    trace = os.environ.get("KERNEL_TRACE", "0") == "1"
    inputs = {k: np.asarray(v) for k, v in inputs.items()}
    edge_index = inputs["edge_index"].astype(np.int64)
    inputs["edge_index"] = edge_index
    for k in list(inputs):
        if inputs[k].dtype == np.float64:
            inputs[k] = inputs[k].astype(np.float32)

    order, tiles_per_win, win_of, pos_of = _shard(edge_index)

    # shared weights (host-side dtype conversion / packing)
    W3b = np.vstack([
        inputs["W3"].astype(np.float32),
        inputs["b3"].astype(np.float32).reshape(1, WNUM),
    ])
    W3p = _w3_permute(W3b).astype(bf16)
    Wn = np.zeros((33, 64), np.float32)
    Wn[0, 0:32] = inputs["bs_s"]
    Wn[1:, 0:32] = inputs["ws_s"]
    Wn[0, 32:64] = inputs["bg_s"]
    Wn[1:, 32:64] = inputs["wg_s"]
    Wv = np.zeros((16, 32), np.float32)
    Wv[:, 0:16] = inputs["ws_v"]
    Wv[:, 16:32] = inputs["wg_v"]
    # one block-diagonal weight so each window's whole self-interaction +
    # gate pre-activation is a single [81,128]@[81,160] matmul
    W_all = np.zeros((81, 160), np.float32)
    W_all[0:33, 0:64] = Wn
    for cc in range(3):
        W_all[33 + cc * 16:49 + cc * 16, 64 + cc * 32:96 + cc * 32] = Wv
    bnw_row = np.zeros((1, 80), np.float32)
    bnw_row[0, 0:32] = inputs["bn_ws"]
    for cc in range(3):
        bnw_row[0, 32 + cc * 16:48 + cc * 16] = inputs["bn_wv"]
    bnb_row = np.zeros((1, 80), np.float32)
    bnb_row[0, 0:32] = inputs["bn_bs"]

    core_maps = []
    T = None
    for c in range(N_CORES):
        m, T = _build_core_inputs(
            inputs, order, tiles_per_win, win_of, pos_of, c, bf16)
        m["W1"] = inputs["W1"].astype(bf16)
        m["b1"] = inputs["b1"].astype(np.float32).reshape(64, 1)
        m["W2"] = inputs["W2"].astype(bf16)
        m["b2"] = inputs["b2"].astype(np.float32).reshape(64, 1)
        m["W3"] = W3p
        m["W_all"] = W_all.astype(bf16)
        m["bnw"] = bnw_row
        m["bnb"] = bnb_row
        core_maps.append(m)

    nc1 = _fused_bass(T, tiles_per_win)
    r1 = run_bass_kernel_spmd(
        nc1, core_maps, core_ids=list(range(N_CORES)), trace=trace)
    global _last_exec_ns
    _last_exec_ns = r1.exec_time_ns
    _last_rr.clear()
    _last_rr.append(r1)

    out = np.zeros((N_NODES, 80), np.float32)
    for c in range(N_CORES):
        slab = r1.results[c]["out"]          # [WIN, N_WIN, 80]
        blk = slab[pos_of[c], win_of[c]]     # [2500, 80] internal layout
        out[c * NPC:(c + 1) * NPC] = blk[:, _perm_int2ref]
    return out


if __name__ == "__main__":
    import reference

    inp = reference.setup_inputs()
    inp = {k: np.asarray(v) for k, v in inp.items()}
    expected = np.asarray(reference.reference(**inp))
    actual = kernel(**inp)
    err = np.abs(actual - expected)
    rel = np.linalg.norm(actual - expected) / np.linalg.norm(expected)
    print("max abs err:", err.max(), "rel:", rel)


# revision 52
# speedup vs baseline: 9.1839x; 1.0484x over previous
"""Trainium2 Bass kernel for nn_EquivariantBlock (gnn_message_passing).

Single fused kernel, nodes partitioned across 8 cores (2500 each).
Host does sharding/gather/layout only; all FLOPs run on device.

Per core: edges grouped by destination window (128-node windows, node->window
assignment load-balanced via LPT so every window needs ~the same tile count).
Edge phase per 128-edge tile: bf16 edge MLP on PE -> per-edge TP weights in
PSUM (never touch HBM; W3 columns pre-permuted + CG/alpha scales folded on
host) -> ACT evacuates weights to SBUF bf16 -> DVE forms per-edge products
(plain tensor_tensor, 2x bf16 mode) -> Pool engine reduces -> one-hot scatter
matmul accumulates per-window sums in PSUM.  Node phase interleaved per
window: degree-mean, self-interaction + gate matmuls (bf16), batch-stat slab.
End phase: stats matmul -> 640B AllReduce across the 8 cores -> BN scale/shift
vectors on-device -> broadcast via matmul -> gated residual update -> one DMA.
"""

import numpy as np

MUL0, MUL1 = 32, 16
EDGE_DIM, HID = 32, 64
WNUM = 2304
N_NODES, N_EDGES = 20000, 100000
EPS = 1e-5
ALPHA = 1.0 / np.sqrt(48.0)
INV_SQRT3 = 1.0 / np.sqrt(3.0)

N_CORES = 8
NPC = N_NODES // N_CORES          # nodes per core = 2500
N_WIN = 20                        # windows per core
NPW = NPC // N_WIN                # nodes per window = 125 (< 128)
WIN = 128                         # window slot size (partition dim)
NPAD = N_WIN * WIN                # 2560
ET = 128                          # edges per tile

_last_exec_ns = None
_last_rr = []

# column permutation: reference h layout [s(32), v (i-major: i*3+c)]
# internal layout  [s(32), v (c-major: c*16+i)]
_perm_ref2int = np.concatenate(
    [np.arange(32)] + [32 + np.arange(16) * 3 + c for c in range(3)]
).astype(np.int64)
_perm_int2ref = np.argsort(_perm_ref2int)


def _w3_permute(W3b):
    """Permute + scale the (W3;b3) columns for the on-device TP layout.

    Input rows [65, 2304] in reference order:
      w00 col i*32+o, w10 1024+i*32+o, w01 1536+i*16+o, w11 2048+i*16+o.

    Output layout, scales folded.  Blocks 00/10/11 use a k-interleaved
    order (i = k*8 + r) so the i-contraction runs as flat contiguous
    half-adds on DVE followed by an 8-wide window-level reduce:
      00 region       k*256 + o*8 + r (k<4,o<32,r<8): alpha*w00[k*8+r, o]
      10 region 1024+ k*256 + o*8 + r (k<2,o<32,r<8): a/sqrt3*w10[k*8+r,o]
      01 region 1536+ o*32 + i       (o<16,i<32):     alpha*w01[i,o]
      11 region 2048+ k*128 + o*8 + r (k<2,o<16,r<8): alpha*w11[k*8+r,o]
    """
    idx = np.empty(WNUM, np.int64)
    scl = np.empty(WNUM, np.float32)
    for k in range(4):
        for o in range(32):
            for r in range(8):
                idx[k * 256 + o * 8 + r] = (k * 8 + r) * 32 + o
                scl[k * 256 + o * 8 + r] = ALPHA
    for k in range(2):
        for o in range(32):
            for r in range(8):
                idx[1024 + k * 256 + o * 8 + r] = 1024 + (k * 8 + r) * 32 + o
                scl[1024 + k * 256 + o * 8 + r] = ALPHA * INV_SQRT3
    for o in range(16):
        for i in range(32):
            idx[1536 + o * 32 + i] = 1536 + i * 16 + o
            scl[1536 + o * 32 + i] = ALPHA
    for k in range(2):
        for o in range(16):
            for r in range(8):
                idx[2048 + k * 128 + o * 8 + r] = 2048 + (k * 8 + r) * 16 + o
                scl[2048 + k * 128 + o * 8 + r] = ALPHA
    return (W3b[:, idx] * scl[None, :]).astype(np.float32)


def _balance_nodes(dst):
    """Assign each core's local nodes to N_WIN windows (<=NPW nodes each),
    balancing per-window edge counts (greedy LPT).  Returns win_of, pos_of
    [N_CORES, NPC] and per-(core,window) edge counts."""
    core = dst // NPC
    dloc = dst - core * NPC
    win_of = np.zeros((N_CORES, NPC), np.int64)
    pos_of = np.zeros((N_CORES, NPC), np.int64)
    ecnt = np.zeros((N_CORES, N_WIN), np.int64)
    for c in range(N_CORES):
        deg = np.bincount(dloc[core == c], minlength=NPC)
        order = np.argsort(-deg, kind="stable")
        loads = np.zeros(N_WIN, np.int64)
        counts = np.zeros(N_WIN, np.int64)
        for n in order:
            open_w = np.nonzero(counts < NPW)[0]
            w = open_w[np.argmin(loads[open_w])]
            win_of[c, n] = w
            pos_of[c, n] = counts[w]
            counts[w] += 1
            loads[w] += deg[n]
        ecnt[c] = loads
    return win_of, pos_of, ecnt


def _shard(edge_index):
    src, dst = edge_index[0], edge_index[1]
    win_of, pos_of, ecnt = _balance_nodes(dst)
    tiles_per_win = [
        max(1, int(max((ecnt[c, w] + ET - 1) // ET for c in range(N_CORES))))
        for w in range(N_WIN)
    ]
    core = dst // NPC
    dloc = dst - core * NPC
    order = [[None] * N_WIN for _ in range(N_CORES)]
    for c in range(N_CORES):
        idx = np.nonzero(core == c)[0]
        w_of = win_of[c][dloc[idx]]
        s = np.argsort(w_of, kind="stable")
        idx = idx[s]
        w_of = w_of[s]
        bounds = np.searchsorted(w_of, np.arange(N_WIN + 1))
        for w in range(N_WIN):
            order[c][w] = idx[bounds[w]:bounds[w + 1]]
    return order, tiles_per_win, win_of, pos_of


def _build_core_inputs(inputs, order, tiles_per_win, win_of, pos_of, c, bf16):
    h = inputs["h"]
    edge_sh = inputs["edge_sh"]
    ef = inputs["edge_features"]
    src = inputs["edge_index"][0]
    dst = inputs["edge_index"][1]

    T = int(sum(tiles_per_win))
    E_pad = T * ET
    svx = np.zeros((E_pad, 96), np.float32)
    efp = np.zeros((E_pad, EDGE_DIM), np.float32)

    pos = 0
    for w in range(N_WIN):
        ids = order[c][w]
        n = len(ids)
        sl = slice(pos, pos + n)
        hs = h[src[ids]][:, _perm_ref2int]     # [n, 80] internal layout
        svx[sl, 0:80] = hs
        svx[sl, 80] = edge_sh[ids, 0]
        svx[sl, 81:84] = edge_sh[ids, 1:4]
        svx[sl, 84] = 1.0
        svx[sl, 85] = -pos_of[c][dst[ids] - c * NPC].astype(np.float32)
        efp[sl] = ef[ids]
        pos += tiles_per_win[w] * ET
    efT = np.ascontiguousarray(
        efp.reshape(T, ET, EDGE_DIM).transpose(0, 2, 1)
    )

    # node-side layouts follow the per-core window permutation
    hsl = h[c * NPC:(c + 1) * NPC][:, _perm_ref2int]   # [2500, 80] internal
    col = win_of[c] * WIN + pos_of[c]                  # node -> slab column
    hT_all = np.zeros((81, NPAD), np.float32)
    hT_all[0, col] = 1.0
    hT_all[1:33, col] = hsl[:, :32].T
    for cc in range(3):
        hT_all[33 + cc * 16:49 + cc * 16, col] = \
            hsl[:, 32 + cc * 16:32 + (cc + 1) * 16].T
    h_nm = np.zeros((WIN, N_WIN, 80), np.float32)      # node-major slab
    h_nm[pos_of[c], win_of[c]] = hsl

    return dict(
        svx=svx.reshape(T, ET, 96).astype(bf16),
        efT=efT.astype(bf16),
        hT_all=hT_all.astype(bf16),
        h_nm=h_nm,
    ), T


def _fused_bass(T, tiles_per_win):
    import concourse.bacc as bacc
    import concourse.mybir as mybir
    import concourse.tile as tile

    fp32 = mybir.dt.float32
    bf16 = mybir.dt.bfloat16
    Alu = mybir.AluOpType
    Act = mybir.ActivationFunctionType
    X = mybir.AxisListType.X

    nc = bacc.Bacc("TRN2", target_bir_lowering=False, debug=False,
                   num_devices=N_CORES)
    d_svx = nc.dram_tensor("svx", [T, ET, 96], bf16, kind="ExternalInput")
    d_efT = nc.dram_tensor("efT", [T, 32, ET], bf16, kind="ExternalInput")
    d_hall = nc.dram_tensor("hT_all", [81, NPAD], bf16,
                            kind="ExternalInput")
    d_hnm = nc.dram_tensor("h_nm", [WIN, N_WIN, 80], fp32,
                           kind="ExternalInput")
    d_W1 = nc.dram_tensor("W1", [32, 64], bf16, kind="ExternalInput")
    d_b1 = nc.dram_tensor("b1", [64, 1], fp32, kind="ExternalInput")
    d_W2 = nc.dram_tensor("W2", [64, 64], bf16, kind="ExternalInput")
    d_b2 = nc.dram_tensor("b2", [64, 1], fp32, kind="ExternalInput")
    d_W3 = nc.dram_tensor("W3", [65, WNUM], bf16, kind="ExternalInput")
    d_wall = nc.dram_tensor("W_all", [81, 160], bf16, kind="ExternalInput")
    d_bnw = nc.dram_tensor("bnw", [1, 80], fp32, kind="ExternalInput")
    d_bnb = nc.dram_tensor("bnb", [1, 80], fp32, kind="ExternalInput")
    d_out = nc.dram_tensor("out", [WIN, N_WIN, 80], fp32,
                           kind="ExternalOutput")

    with tile.TileContext(nc) as tc, \
            nc.allow_low_precision(reason="bf16 TP well within 2e-2 tol"):
        with tc.tile_pool(name="singles", bufs=1) as singles:
            sW1 = singles.tile([32, 64], bf16)
            nc.sync.dma_start(out=sW1, in_=d_W1[:, :])
            sb1 = singles.tile([64, 1], fp32)
            nc.sync.dma_start(out=sb1, in_=d_b1[:, :])
            sW2 = singles.tile([64, 64], bf16)
            nc.sync.dma_start(out=sW2, in_=d_W2[:, :])
            sb2 = singles.tile([64, 1], fp32)
            nc.sync.dma_start(out=sb2, in_=d_b2[:, :])
            sW3 = singles.tile([65, WNUM], bf16)
            nc.sync.dma_start(out=sW3, in_=d_W3[:, :])
            sWall = singles.tile([81, 160], bf16)
            nc.sync.dma_start(out=sWall, in_=d_wall[:, :])
            sHall = singles.tile([81, NPAD], bf16)
            nc.sync.dma_start(out=sHall, in_=d_hall[:, :])
            # h_nm is only read in the end phase; DMA issued there so the
            # 820KB transfer doesn't stall the first edge tiles' loads
            h_nm = singles.tile([WIN, N_WIN, 80], fp32)
            bnw = singles.tile([1, 80], fp32)
            nc.sync.dma_start(out=bnw, in_=d_bnw[:, :])
            bnb = singles.tile([1, 80], fp32)
            nc.sync.dma_start(out=bnb, in_=d_bnb[:, :])

            iotaI = singles.tile([ET, WIN], mybir.dt.int32)
            nc.gpsimd.iota(iotaI, [[1, WIN]], channel_multiplier=0)
            iotaB = singles.tile([ET, WIN], bf16)
            nc.vector.tensor_copy(iotaB, iotaI)
            ones_col = singles.tile([WIN, 1], bf16)
            nc.vector.memset(ones_col, 1.0)
            ones_row = singles.tile([1, WIN], fp32)
            nc.vector.memset(ones_row, 1.0)

            slab_st = singles.tile([WIN, N_WIN, 160], bf16)
            slab_g = singles.tile([WIN, N_WIN, 80], bf16)
            slab_out = singles.tile([WIN, N_WIN, 80], fp32)

            # manual double-buffer for x2a: row 64 is a constant ones row
            # (the b3 contraction row), written once — pools would force a
            # per-tile rewrite.
            x2a_bufs = []
            for i in range(2):
                xt = singles.tile([65, ET], bf16, tag=f"x2a{i}")
                nc.vector.memset(xt[64:65, :], 1.0)
                x2a_bufs.append(xt)

            # self-interaction + gate pre-acts for ALL windows, in a
            # dedicated pre-scope with 4 PSUM banks (free before the edge
            # pools open) so the 20 matmuls pipeline instead of ping-ponging
            # through one bank, overlapping the edge-phase DMA fill.
            # Layout per window: [si_s(32)|gate_s(32)|3 x (si_v(16)|gate_v(16))]
            slab_nv = singles.tile([WIN, N_WIN, 160], fp32)
            with tc.tile_pool(name="hps", bufs=4, space="PSUM") as hps:
                for w in range(N_WIN):
                    nmo = hps.tile([128, 160], fp32, tag="hmm")
                    nc.tensor.matmul(nmo,
                                     sHall[:, w * WIN:(w + 1) * WIN], sWall,
                                     start=True, stop=True)
                    nc.scalar.activation(slab_nv[:, w, :], nmo, Act.Copy)

            # ---------------- edge + node phase (per window) ----------------
            # scatter payload layout (945 bf16 cols per edge):
            #   0:256    00-block partial sums, (o=32, r=8)
            #   256:512  10-block partial sums, (o=32, r=8)
            #   512:896  11-block partial sums, (c=3, o=16, r=8)
            #   896:944  p01*sh1, (c=3, o=16)
            #   944      valid (degree)
            with (
                tc.tile_pool(name="edma", bufs=6) as edma,
                tc.tile_pool(name="esb", bufs=4) as esb,
                tc.tile_pool(name="nsb", bufs=2) as nsb,
                tc.tile_pool(name="wpsA", bufs=1, space="PSUM") as wpsA,
                tc.tile_pool(name="wps01", bufs=1, space="PSUM") as wps01,
                tc.tile_pool(name="wpsB", bufs=1, space="PSUM") as wpsB,
                tc.tile_pool(name="mmout", bufs=1, space="PSUM") as mmout,
                tc.tile_pool(name="sps", bufs=1, space="PSUM") as sps,
            ):
                t_idx = 0
                for w in range(N_WIN):
                    ps_sum = sps.tile([WIN, 945], fp32, tag="scat")
                    jlast = tiles_per_win[w] - 1
                    for j in range(tiles_per_win[w]):
                        t = t_idx
                        t_idx += 1
                        sv = edma.tile([ET, 96], bf16, tag="svx")
                        nc.sync.dma_start(out=sv, in_=d_svx[t, :, :])
                        ef_t = edma.tile([32, ET], bf16, tag="ef")
                        nc.sync.dma_start(out=ef_t, in_=d_efT[t, :, :])

                        # --- edge MLP (feature-major, bf16) ---
                        mo1 = mmout.tile([128, 160], fp32, tag="mm")
                        nc.tensor.matmul(mo1[0:64, 0:128], sW1, ef_t,
                                         start=True, stop=True)
                        x1 = esb.tile([64, ET], bf16, tag="x1")
                        nc.scalar.activation(x1, mo1[0:64, 0:128], Act.Silu,
                                             bias=sb1)
                        mo2 = mmout.tile([128, 160], fp32, tag="mm")
                        nc.tensor.matmul(mo2[0:64, 0:128], sW2, x1,
                                         start=True, stop=True)
                        x2a = x2a_bufs[t % 2]
                        nc.scalar.activation(x2a[0:64, :], mo2[0:64, 0:128],
                                             Act.Silu, bias=sb2)

                        # --- mm3: per-edge TP weights, 3 PSUM regions ---
                        psA = wpsA.tile([ET, 1536], fp32, tag="A")
                        for c0 in (0, 512, 1024):
                            nc.tensor.matmul(psA[:, c0:c0 + 512], x2a,
                                             sW3[:, c0:c0 + 512],
                                             start=True, stop=True)
                        ps01 = wps01.tile([ET, 512], fp32, tag="o1")
                        nc.tensor.matmul(ps01, x2a, sW3[:, 1536:2048],
                                         start=True, stop=True)
                        psB = wpsB.tile([ET, 256], fp32, tag="B")
                        nc.tensor.matmul(psB, x2a, sW3[:, 2048:2304],
                                         start=True, stop=True)

                        # --- per-edge features ---
                        # fp32 copies of the per-edge scalars (ts needs f32)
                        aux32 = esb.tile([ET, 6], fp32, tag="aux32")
                        nc.gpsimd.tensor_copy(aux32, sv[:, 80:86])
                        # fAV = [se*sh0 (32) | dv (16) | vec*sh0 (48)]
                        # scale-by-partition-scalar runs on ACT (idle)
                        fAV = esb.tile([ET, 96], bf16, tag="fAV")
                        nc.scalar.activation(
                            fAV[:, 0:32], sv[:, 0:32], Act.Copy,
                            scale=aux32[:, 0:1])
                        nc.scalar.activation(
                            fAV[:, 48:96], sv[:, 32:80], Act.Copy,
                            scale=aux32[:, 0:1])
                        t3 = esb.tile([ET, 48], bf16, tag="t3")
                        nc.gpsimd.tensor_tensor(
                            out=t3, in0=sv[:, 32:80],
                            in1=sv[:, 81:84].unsqueeze(2).broadcast_to(
                                (ET, 3, 16)),
                            op=Alu.mult)
                        nc.vector.tensor_reduce(
                            out=fAV[:, 32:48],
                            in_=t3.rearrange("p (c i) -> p i c", c=3),
                            axis=X, op=Alu.add)

                        # --- TP products: DVE straight from PSUM ---
                        prod00 = esb.tile([ET, 1024], bf16, tag="prod00")
                        nc.vector.tensor_tensor(
                            out=prod00.rearrange("p (k o r) -> p k o r",
                                                 k=4, o=32),
                            in0=psA[:, 0:1024].rearrange(
                                "p (k o r) -> p k o r", k=4, o=32),
                            in1=fAV[:, 0:32]
                                .rearrange("p (k r) -> p k r", k=4)
                                .unsqueeze(2).broadcast_to((ET, 4, 32, 8)),
                            op=Alu.mult)
                        prod10 = esb.tile([ET, 512], bf16, tag="prod10")
                        nc.vector.tensor_tensor(
                            out=prod10.rearrange("p (k o r) -> p k o r",
                                                 k=2, o=32),
                            in0=psA[:, 1024:1536].rearrange(
                                "p (k o r) -> p k o r", k=2, o=32),
                            in1=fAV[:, 32:48]
                                .rearrange("p (k r) -> p k r", k=2)
                                .unsqueeze(2).broadcast_to((ET, 2, 32, 8)),
                            op=Alu.mult)
                        fVv = fAV[:, 48:96].rearrange("p (c i) -> p c i",
                                                      c=3)
                        prodBk = []
                        for k in range(2):
                            pk = esb.tile([ET, 384], bf16, tag=f"prodB{k}")
                            nc.vector.tensor_tensor(
                                out=pk.rearrange("p (c o r) -> p c o r",
                                                 c=3, o=16),
                                in0=psB[:, k * 128:(k + 1) * 128]
                                    .rearrange("p (o r) -> p o r", o=16)
                                    .unsqueeze(1)
                                    .broadcast_to((ET, 3, 16, 8)),
                                in1=fVv[:, :, k * 8:(k + 1) * 8]
                                    .unsqueeze(2)
                                    .broadcast_to((ET, 3, 16, 8)),
                                op=Alu.mult)
                            prodBk.append(pk)
                        w01 = esb.tile([ET, 512], bf16, tag="w01")
                        nc.scalar.activation(w01, ps01, Act.Copy)
                        prod01 = esb.tile([ET, 512], bf16, tag="prod01")
                        nc.gpsimd.tensor_tensor(
                            out=prod01.rearrange("p (o i) -> p o i", o=16),
                            in0=w01.rearrange("p (o i) -> p o i", o=16),
                            in1=sv[:, 0:32].unsqueeze(1).broadcast_to(
                                (ET, 16, 32)),
                            op=Alu.mult)

                        # --- fold k-halves: flat contiguous adds ---
                        msgw = esb.tile([ET, 945], bf16, tag="msgw")
                        t00 = esb.tile([ET, 512], bf16, tag="t00")
                        nc.vector.tensor_tensor(
                            out=t00, in0=prod00[:, 0:512],
                            in1=prod00[:, 512:1024], op=Alu.add)
                        nc.vector.tensor_tensor(
                            out=msgw[:, 0:256], in0=t00[:, 0:256],
                            in1=t00[:, 256:512], op=Alu.add)
                        nc.gpsimd.tensor_tensor(
                            out=msgw[:, 256:512], in0=prod10[:, 0:256],
                            in1=prod10[:, 256:512], op=Alu.add)
                        nc.gpsimd.tensor_tensor(
                            out=msgw[:, 512:896], in0=prodBk[0],
                            in1=prodBk[1], op=Alu.add)

                        # --- 01-block: full reduce + sh1 outer product ---
                        p01 = esb.tile([ET, 16], bf16, tag="p01")
                        nc.vector.tensor_reduce(
                            out=p01,
                            in_=prod01.rearrange("p (o i) -> p o i", o=16),
                            axis=X, op=Alu.add)
                        nc.gpsimd.tensor_tensor(
                            out=msgw[:, 896:944].rearrange(
                                "p (c o) -> p c o", c=3),
                            in0=p01.unsqueeze(1).broadcast_to((ET, 3, 16)),
                            in1=sv[:, 81:84].unsqueeze(2).broadcast_to(
                                (ET, 3, 16)),
                            op=Alu.mult)
                        nc.gpsimd.tensor_copy(msgw[:, 944:945], sv[:, 84:85])

                        # --- one-hot scatter matmul (moving dim <= 512) ---
                        # one-hot on ACT: relu(1 - |iota - dstw|); svx col 85
                        # holds -dstw so it can ride the activation bias
                        absd = esb.tile([ET, WIN], bf16, tag="absd")
                        nc.scalar.activation(absd, iotaB, Act.Abs,
                                             bias=aux32[:, 5:6])
                        S = esb.tile([ET, WIN], bf16, tag="S")
                        nc.scalar.activation(S, absd, Act.Relu,
                                             scale=-1.0, bias=1.0)
                        for c0, c1 in ((0, 512), (512, 945)):
                            nc.tensor.matmul(
                                ps_sum[:, c0:c1], S, msgw[:, c0:c1],
                                start=(j == 0), stop=(j == jlast),
                                skip_group_check=True)

                    # ---------------- node phase for window w ----------------
                    # window-level reduce of the scattered 8-wide partials
                    tw = nsb.tile([WIN, 112], fp32, tag="tw")
                    nc.vector.tensor_reduce(
                        out=tw,
                        in_=ps_sum[:, 0:896].rearrange(
                            "p (g r) -> p g r", g=112),
                        axis=X, op=Alu.add)
                    summed = nsb.tile([WIN, 80], fp32, tag="summed")
                    nc.vector.tensor_tensor(
                        out=summed[:, 0:32], in0=tw[:, 0:32],
                        in1=tw[:, 32:64], op=Alu.add)
                    nc.vector.tensor_tensor(
                        out=summed[:, 32:80], in0=tw[:, 64:112],
                        in1=ps_sum[:, 896:944], op=Alu.add)
                    degc = nsb.tile([WIN, 1], fp32, tag="degc")
                    nc.vector.tensor_scalar(
                        degc, ps_sum[:, 944:945], 1.0, None, op0=Alu.max)
                    rdeg = nsb.tile([WIN, 1], fp32, tag="rdeg")
                    nc.vector.reciprocal(rdeg, degc)
                    agg = nsb.tile([WIN, 80], fp32, tag="agg")
                    nc.vector.tensor_scalar(
                        agg, summed, rdeg, None, op0=Alu.mult)

                    # upd -> slab_st[:, w, 0:80]; sq -> [:, w, 80:160]
                    # (self-interaction comes from the SBUF slab, so these
                    # run on Pool; sigmoids are batched in the end phase)
                    nv_v = slab_nv[:, w, 64:160].rearrange(
                        "p (c k) -> p c k", c=3)
                    nc.gpsimd.tensor_tensor(
                        out=slab_st[:, w, 0:32], in0=agg[:, 0:32],
                        in1=slab_nv[:, w, 0:32], op=Alu.add)
                    nc.gpsimd.tensor_tensor(
                        out=slab_st[:, w, 32:80].rearrange(
                            "p (c i) -> p c i", c=3),
                        in0=agg[:, 32:80].rearrange("p (c i) -> p c i", c=3),
                        in1=nv_v[:, :, 0:16], op=Alu.add)
                    nc.gpsimd.tensor_tensor(
                        out=slab_st[:, w, 80:160],
                        in0=slab_st[:, w, 0:80],
                        in1=slab_st[:, w, 0:80], op=Alu.mult)

            # ---------------- end phase: stats, allreduce, BN, update -------
            with (
                tc.tile_pool(name="eps", bufs=1, space="PSUM") as eps_p,
                tc.tile_pool(name="fsb", bufs=1) as fsb,
                tc.tile_pool(name="dram", bufs=2, space="DRAM") as dram,
            ):
                nc.sync.dma_start(out=h_nm, in_=d_hnm[:, :, :])
                # all gate sigmoids in two batched instrs (one table load)
                nc.scalar.activation(
                    slab_g[:, :, 0:32], slab_nv[:, :, 32:64], Act.Sigmoid)
                nc.scalar.activation(
                    slab_g[:, :, 32:80].rearrange(
                        "p w (c i) -> p w c i", c=3),
                    slab_nv[:, :, 64:160].rearrange(
                        "p w (c k) -> p w c k", c=3)[:, :, :, 16:32],
                    Act.Sigmoid)
                ps_st = eps_p.tile([1, 160], fp32, tag="st")
                for w in range(N_WIN):
                    nc.tensor.matmul(
                        ps_st, ones_col, slab_st[:, w, :],
                        start=(w == 0), stop=(w == N_WIN - 1),
                        skip_group_check=True)
                st_sb = fsb.tile([1, 160], fp32, tag="stsb")
                nc.scalar.activation(st_sb, ps_st, Act.Copy)

                ib = dram.tile([1, 160], fp32, tag="ib")
                ob = dram.tile([1, 160], fp32, tag="ob")
                nc.gpsimd.dma_start(ib[:], st_sb[:])
                nc.gpsimd.collective_compute(
                    "AllReduce", mybir.AluOpType.add,
                    replica_groups=[list(range(N_CORES))],
                    ins=[ib.opt()], outs=[ob.opt()])
                st_r = fsb.tile([1, 160], fp32, tag="str")
                nc.gpsimd.dma_start(st_r[:], ob[:])

                inv_n = 1.0 / float(N_NODES)
                meanb = fsb.tile([1, 80], fp32, tag="meanb")
                nc.vector.tensor_scalar(
                    meanb, st_r[:, 0:80], inv_n, None, op0=Alu.mult)
                nc.vector.memset(meanb[:, 32:80], 0.0)
                ex2 = fsb.tile([1, 80], fp32, tag="ex2")
                nc.vector.tensor_scalar(
                    ex2, st_r[:, 80:160], inv_n, None, op0=Alu.mult)
                m2 = fsb.tile([1, 80], fp32, tag="m2")
                nc.vector.tensor_tensor(out=m2, in0=meanb, in1=meanb,
                                        op=Alu.mult)
                exm = fsb.tile([1, 80], fp32, tag="exm")
                nc.vector.tensor_tensor(out=exm, in0=ex2, in1=m2,
                                        op=Alu.subtract)
                vn = fsb.tile([1, 16], fp32, tag="vn")
                nc.vector.tensor_reduce(
                    out=vn,
                    in_=exm[:, 32:80].rearrange("p (c i) -> p i c", c=3),
                    axis=X, op=Alu.add)
                varb = fsb.tile([1, 80], fp32, tag="varb")
                nc.vector.tensor_scalar(
                    varb[:, 0:32], exm[:, 0:32], 1.0, float(EPS),
                    op0=Alu.mult, op1=Alu.add)
                nc.vector.tensor_scalar(
                    varb[:, 32:80].rearrange("p (c i) -> p c i", c=3),
                    vn.unsqueeze(1).broadcast_to((1, 3, 16)),
                    1.0 / 3.0, float(EPS), op0=Alu.mult, op1=Alu.add)
                rec = fsb.tile([1, 80], fp32, tag="rec")
                nc.vector.reciprocal(rec, varb)
                rstd = fsb.tile([1, 80], fp32, tag="rstd")
                nc.scalar.activation(rstd, rec, Act.Sqrt)
                scsh = fsb.tile([1, 160], fp32, tag="scsh")
                nc.vector.tensor_tensor(
                    out=scsh[:, 0:80], in0=rstd, in1=bnw, op=Alu.mult)
                msc = fsb.tile([1, 80], fp32, tag="msc")
                nc.vector.tensor_tensor(
                    out=msc, in0=meanb, in1=scsh[:, 0:80], op=Alu.mult)
                nc.vector.tensor_tensor(
                    out=scsh[:, 80:160], in0=bnb, in1=msc, op=Alu.subtract)

                ps_b = eps_p.tile([128, 160], fp32, tag="bc")
                nc.tensor.matmul(ps_b, ones_row, scsh, start=True, stop=True)
                scshB = fsb.tile([128, 160], fp32, tag="scshB")
                nc.scalar.activation(scshB, ps_b, Act.Copy)

                # batched gated residual update over the whole node slab:
                # out = (upd*sc + sh) * g + h, broadcasting sc/sh per window
                scB = scshB[:, 0:80].unsqueeze(1).broadcast_to(
                    (WIN, N_WIN, 80))
                shB = scshB[:, 80:160].unsqueeze(1).broadcast_to(
                    (WIN, N_WIN, 80))
                t1 = fsb.tile([WIN, N_WIN, 80], fp32, tag="t1")
                nc.vector.tensor_tensor(
                    out=t1, in0=slab_st[:, :, 0:80], in1=scB, op=Alu.mult)
                t2 = fsb.tile([WIN, N_WIN, 80], fp32, tag="t2")
                nc.vector.tensor_tensor(
                    out=t2, in0=t1, in1=shB, op=Alu.add)
                nc.vector.tensor_tensor(
                    out=t1, in0=t2, in1=slab_g, op=Alu.mult)
                nc.vector.tensor_tensor(
                    out=slab_out, in0=t1, in1=h_nm, op=Alu.add)
                nc.sync.dma_start(out=d_out[:, :, :], in_=slab_out)
    nc.compile()
    return nc


def kernel(**inputs):
    import os
    from concourse.bass_utils import run_bass_kernel_spmd
    import ml_dtypes

    bf16 = ml_dtypes.bfloat16
    trace = os.environ.get("KERNEL_TRACE", "0") == "1"
    inputs = {k: np.asarray(v) for k, v in inputs.items()}
    edge_index = inputs["edge_index"].astype(np.int64)
    inputs["edge_index"] = edge_index
    for k in list(inputs):
        if inputs[k].dtype == np.float64:
            inputs[k] = inputs[k].astype(np.float32)

    order, tiles_per_win, win_of, pos_of = _shard(edge_index)

    # shared weights (host-side dtype conversion / packing)
    W3b = np.vstack([
        inputs["W3"].astype(np.float32),
        inputs["b3"].astype(np.float32).reshape(1, WNUM),
    ])
    W3p = _w3_permute(W3b).astype(bf16)
    Wn = np.zeros((33, 64), np.float32)
    Wn[0, 0:32] = inputs["bs_s"]
    Wn[1:, 0:32] = inputs["ws_s"]
    Wn[0, 32:64] = inputs["bg_s"]
    Wn[1:, 32:64] = inputs["wg_s"]
    Wv = np.zeros((16, 32), np.float32)
    Wv[:, 0:16] = inputs["ws_v"]
    Wv[:, 16:32] = inputs["wg_v"]
    # one block-diagonal weight so each window's whole self-interaction +
    # gate pre-activation is a single [81,128]@[81,160] matmul
    W_all = np.zeros((81, 160), np.float32)
    W_all[0:33, 0:64] = Wn
    for cc in range(3):
        W_all[33 + cc * 16:49 + cc * 16, 64 + cc * 32:96 + cc * 32] = Wv
    bnw_row = np.zeros((1, 80), np.float32)
    bnw_row[0, 0:32] = inputs["bn_ws"]
    for cc in range(3):
        bnw_row[0, 32 + cc * 16:48 + cc * 16] = inputs["bn_wv"]
    bnb_row = np.zeros((1, 80), np.float32)
    bnb_row[0, 0:32] = inputs["bn_bs"]

    core_maps = []
    T = None
    for c in range(N_CORES):
        m, T = _build_core_inputs(
            inputs, order, tiles_per_win, win_of, pos_of, c, bf16)
        m["W1"] = inputs["W1"].astype(bf16)
        m["b1"] = inputs["b1"].astype(np.float32).reshape(64, 1)
        m["W2"] = inputs["W2"].astype(bf16)
        m["b2"] = inputs["b2"].astype(np.float32).reshape(64, 1)
        m["W3"] = W3p
        m["W_all"] = W_all.astype(bf16)
        m["bnw"] = bnw_row
        m["bnb"] = bnb_row
        core_maps.append(m)

    nc1 = _fused_bass(T, tiles_per_win)
    r1 = run_bass_kernel_spmd(
        nc1, core_maps, core_ids=list(range(N_CORES)), trace=trace)
    global _last_exec_ns
    _last_exec_ns = r1.exec_time_ns
    _last_rr.clear()
    _last_rr.append(r1)

    out = np.zeros((N_NODES, 80), np.float32)
    for c in range(N_CORES):
        slab = r1.results[c]["out"]          # [WIN, N_WIN, 80]
        blk = slab[pos_of[c], win_of[c]]     # [2500, 80] internal layout
        out[c * NPC:(c + 1) * NPC] = blk[:, _perm_int2ref]
    return out


if __name__ == "__main__":
    import reference

    inp = reference.setup_inputs()
    inp = {k: np.asarray(v) for k, v in inp.items()}
    expected = np.asarray(reference.reference(**inp))
    actual = kernel(**inp)
    err = np.abs(actual - expected)
    rel = np.linalg.norm(actual - expected) / np.linalg.norm(expected)
    print("max abs err:", err.max(), "rel:", rel)
